# revision 1
# baseline (speedup 1.0000x reference)
"""Trainium2 Bass kernel for ALiBiConformerEncoderLayer (8-core SPMD).

Sharding: sequence windows (256 queries + 1 halo col each side) per core.

Key ideas vs the v0 baseline:
- ALiBi is folded into the scores matmul itself. For keys outside the query
  window, exp(alibi) = exp(-slope*(edgedist(sk) + coldist(f))) factorizes into
  a per-sk row term and a per-f column term. Extra contraction rows carry it:
  kT gets per-head rows B_h[sk] = -slope_h*edgedist(sk) plus two indicator
  rows [sk<q0], [sk>q0+257]; the q-side slot tile gets a ones row and
  -slope_h*f / -slope_h*(257-f) rows. attn = exp(scores) directly - no
  17MB/core ealibi DMA, no per-block elementwise multiply. Only the 4 blocks
  intersecting the window need a (host-prepped, bias-compensated) 2D
  exp(alibi) multiply.
- Per-head-pair block skipping: blocks beyond distance T/slope contribute
  < e^-T and are dropped. Heads are processed in pairs; slots per pair are
  compile-time constants. Core-dependence is handled by a host-side
  window-centered block permutation (17 slots; dummies have masked v).
- kT chunks hold 6/6/4 heads (16 rows each) + bias/indicator rows.
- Software-pipelined attention loop keeps PE dense (HAM stays warm).
- LN rsqrt via Ln/Exp (same activation table set as the attention exp),
  LN affine on DVE: minimal activation-table switches.
"""
import os
import sys
import types
from contextlib import ExitStack

import numpy as np
import ml_dtypes

BF16 = ml_dtypes.bfloat16

# Problem constants (hardcoded; kernel.py must be self-contained)
B, S, D, H, HD = 2, 2048, 256, 16, 16
NCORES = 8
WIN = S // NCORES          # 256
SQL = WIN + 2              # query cols incl 1 halo each side
SQP = 260                  # 4-elem aligned bf16 stride
NB = S // 128              # 16 real sk blocks
NSLOT = 17                 # 16 real + 1 dummy (window-centered permutation)
SK = NSLOT * 128

# kT chunking: heads per chunk, base head of chunk
NH = [6, 6, 4]
HBASE = [0, 6, 12]
CHUNK_OF_HEAD = [0] * 6 + [1] * 6 + [2] * 4

# Block-skip: pair pi covers heads (2pi, 2pi+1); reach D = T/slope(2pi+1).
# slopes 2^-(h+1)/2 -> D_pi = T * 2^(pi+1);  T = 13 nats.
# n_p = min(17, 4 + 2*floor(D/128))
N_P = [4, 4, 4, 6, 10, 16, 17, 17]

_COMPILED = {}


def _ensure_ntff_hook():
    """Install the axon NTFF profiling hook if the image lacks antenv.axon_hooks."""
    try:
        import antenv.axon_hooks  # noqa: F401
        return
    except ImportError:
        pass
    try:
        from trn_agent_boot.trn_boot import _ntff_profile_via_ctypes
        hook = _ntff_profile_via_ctypes('/opt/axon/libaxon_pjrt.so')
    except Exception:
        hook = None
    mod = types.ModuleType('antenv.axon_hooks')
    mod.get_axon_ntff_profile_hook = lambda: hook
    mod.set_axon_ntff_profile_hook = lambda h: None
    sys.modules['antenv.axon_hooks'] = mod


# ---------------------------------------------------------------------------
# Graph builder
# ---------------------------------------------------------------------------

def build_nc():
    import concourse.bass as bass  # noqa: F401
    import concourse.tile as tile
    from concourse import bacc, mybir
    from concourse.bass import ts

    f32 = mybir.dt.float32
    f32r = mybir.dt.float32r
    bf16 = mybir.dt.bfloat16
    AF = mybir.ActivationFunctionType
    OP = mybir.AluOpType

    nc = bacc.Bacc(None, target_bir_lowering=False)

    P = {}
    def dram(name, shape, dt):
        P[name] = nc.declare_dram_parameter(name, list(shape), dt, isOutput=False)
        return P[name]

    srckv = dram("srckv", [B, D + 1, SK], bf16)     # permuted src.T + ones row
    srcqbf = dram("srcqbf", [B, D, SQL], bf16)
    srcqf = dram("srcqf", [B, D, SQL], f32)
    wq2 = dram("wq2", [2, D, 3, 96], bf16)          # wqT cols + swapped, chunked
    wk2 = dram("wk2", [2, D, 3, 96], bf16)
    wv = dram("wv", [D + 1, D], bf16)               # wvT + bv row
    ropecs = dram("ropecs", [8, 2, SK], bf16)       # cosT, sinT (k side, permuted)
    ropecsq = dram("ropecsq", [8, 2, SQL], bf16)    # 0.25*cos/sin (q side)
    ropem = dram("ropem", [8, 2, 128], bf16)        # Mcos, Msin(signed)
    qkb = dram("qkb", [128, 3, 4], f32)             # bq, bqs, bk, bks chunk rows
    maskv = dram("maskv", [128, B, NSLOT], f32)     # (1 - mask) permuted
    kTbias = dram("kTbias", [3, 8, SK], bf16)       # B rows (0:NHc) + ind (6:8)
    cq = dram("cq", [128, 16, SQP], bf16)           # q-side slot constant tiles
    eald = dram("eald", [8, 128, 4, 2, SQP], bf16)  # diag exp(alibi)/bias
    tailw = dram("tailw", [5, D, D], f32r)          # woT pw1T pw2T w1T w2T
    tailv = dram("tailv", [128, 2, 16], f32)
    halom = dram("halom", [1, SQL], f32r)
    pmask = dram("pmask", [128, 16], f32)           # head-slot row masks
    onesr = dram("onesr", [128, 128], f32r)
    wop = dram("wop", [4 * 128, D], f32r)   # woT rows in 32-row head slots
    out = nc.declare_dram_parameter("out", [B, D, WIN], f32, isOutput=True)

    with ExitStack() as top, tile.TileContext(nc) as tc:
        _keep = []
        def ctile(shape, dt, name):
            t, _free = tc.tile(list(shape), dt, name=name)
            _keep.append((t, _free))
            return t

        sync = nc.sync

        # ---- persistent SBUF ----
        tw_sb = ctile([128, 5, 2, D], f32r, "tw_sb")
        tv_sb = ctile([128, 2, 16], f32, "tv_sb")
        qkb_sb = ctile([128, 3, 4], f32, "qkb_sb")
        mv_sb = ctile([128, B, NSLOT], f32, "mv_sb")
        hm_sb = ctile([1, SQL], f32r, "hm_sb")
        qsrcf_sb = ctile([128, B, 2, SQL], f32, "qsrcf_sb")
        pm_sb = ctile([128, 16], f32, "pm_sb")
        cq_sb = ctile([128, 16, SQP], bf16, "cq_sb")
        onesr_sb = ctile([128, 128], f32r, "onesr_sb")

        for w in range(5):
            for cc in range(2):
                sync.dma_start(tw_sb[:, w, cc, :], tailw[w, ts(cc, 128), :])
        sync.dma_start(tv_sb[:, :, :], tailv[:, :, :])
        sync.dma_start(qkb_sb[:, :, :], qkb[:, :, :])
        sync.dma_start(mv_sb[:, :, :], maskv[:, :, :])
        sync.dma_start(hm_sb[:, :], halom[:, :])
        sync.dma_start(pm_sb[:, :], pmask[:, :])
        sync.dma_start(cq_sb[:, :, :], cq[:, :, :])
        sync.dma_start(onesr_sb[:, :], onesr[:, :])
        wop_sb = ctile([128, 4, D], f32r, "wop_sb")
        for c4 in range(4):
            sync.dma_start(wop_sb[:, c4, :], wop[ts(c4, 128), :])
        for b in range(B):
            for cc in range(2):
                sync.dma_start(qsrcf_sb[:, b, cc, :], srcqf[b, ts(cc, 128), :])

        ones32 = onesr_sb[0:1, 0:32]
        ones128 = onesr_sb[:, 0:1]
        onesB = onesr_sb[0:1, :]

        onescol = ctile([128, 16, 1], bf16, "onescol")
        nc.vector.memset(onescol[:, :, :], 1.0)
        eps1 = ctile([1, 1], f32, "eps1")
        nc.vector.memset(eps1[:, :], 1e-5)

        # persistent activation tensors
        kT_sb = ctile([128, B, 3, SK], bf16, "kT_sb")
        qTv_sb = ctile([128, 16, B, SQP], bf16, "qTv_sb")
        vt_sb = ctile([128, B, NSLOT, 16, 33], bf16, "vt_sb")
        onum_sb = ctile([33, 16, 2, SQP], bf16, "onum_sb")
        oall_sb = ctile([128, B, 4, SQL], f32r, "oall_sb")
        denp_sb = ctile([32, SQL], bf16, "denp_sb")
        recp_sb = ctile([32, SQL], f32, "recp_sb")
        maskB_sb = ctile([128, SQL], f32, "maskB_sb")

        # NaN-safety for never-written kT/qT rows (multiplied by 0 masks)
        for b in range(B):
            for ch in range(3):
                nc.gpsimd.memset(kT_sb[96:128, b, ch, :], 0.0)

        # ========== PHASE 1+2: prologue interleaved with attention ==========
        # Emit per-chunk: qproj/kproj/vproj for chunk c, then that chunk's
        # attention pairs - prologue DVE work hides under exp-bound attention.
        with ExitStack() as ph1:
            pro = ph1.enter_context(
                tc.tile_pool(name="pro_psum", bufs=2, space="PSUM"))
            ptmp = ph1.enter_context(tc.tile_pool(name="pro_tmp", bufs=6))
            p1c = ph1.enter_context(tc.tile_pool(name="p1c", bufs=1))
            peal = ph1.enter_context(tc.tile_pool(name="peal", bufs=2))
            pscore = ph1.enter_context(
                tc.tile_pool(name="pscore", bufs=2, space="PSUM"))
            po_pool = ph1.enter_context(
                tc.tile_pool(name="po", bufs=1, space="PSUM"))
            pexp = ph1.enter_context(tc.tile_pool(name="pexp", bufs=3))
            pattn = ph1.enter_context(tc.tile_pool(name="pattn", bufs=2))

            def p1tile(shape, dt, name):
                return p1c.tile(list(shape), dt, name=name, tag=name)

            wq_sb = p1tile([128, 2, 2, 3, 96], bf16, "wq_sb")
            wk_sb = p1tile([128, 2, 2, 3, 96], bf16, "wk_sb")
            wv_sb = p1tile([128, 2, D], bf16, "wv_sb")
            wvb_sb = p1tile([1, D], bf16, "wvb_sb")
            rm_sb = p1tile([8, 2, 128], bf16, "rm_sb")
            rcs_sb = p1tile([8, 2, SK], bf16, "rcs_sb")
            rcsq_sb = p1tile([8, 2, SQL], bf16, "rcsq_sb")
            kv_sb = p1tile([128, B, 2, SK], bf16, "kv_sb")
            kvo_sb = p1tile([1, B, SK], bf16, "kvo_sb")
            qsrc_sb = p1tile([128, B, 2, SQL], bf16, "qsrc_sb")
            cs_sb = p1tile([128, 2, SK], bf16, "cs_sb")
            csq_sb = p1tile([128, 2, SQL], bf16, "csq_sb")
            qT_sb = p1tile([128, B, 3, SQL], bf16, "qT_sb")

            for b in range(B):
                for ch in range(3):
                    nc.gpsimd.memset(qT_sb[96:128, b, ch, :], 0.0)

            for v in range(2):
                for cc in range(2):
                    sync.dma_start(wq_sb[:, v, cc, :, :],
                                   wq2[v, ts(cc, 128), :, :])
                    sync.dma_start(wk_sb[:, v, cc, :, :],
                                   wk2[v, ts(cc, 128), :, :])
            for cc in range(2):
                sync.dma_start(wv_sb[:, cc, :], wv[ts(cc, 128), :])
            sync.dma_start(wvb_sb[:, :], wv[D:D + 1, :])
            sync.dma_start(rm_sb[:, :, :], ropem[:, :, :])
            sync.dma_start(rcs_sb[:, :, :], ropecs[:, :, :])
            sync.dma_start(rcsq_sb[:, :, :], ropecsq[:, :, :])
            for b in range(B):
                for cc in range(2):
                    sync.dma_start(kv_sb[:, b, cc, :], srckv[b, ts(cc, 128), :])
                    sync.dma_start(qsrc_sb[:, b, cc, :],
                                   srcqbf[b, ts(cc, 128), :])
                sync.dma_start(kvo_sb[:, b, :], srckv[b, D:D + 1, :])

            # rope broadcast tiles: cs[r, t] = M[., r] rows x cosT/sinT
            FCS = [(0, 512), (512, 512), (1024, 512), (1536, 512), (2048, 128)]
            for v in range(2):
                for f0, fw in FCS:
                    pb = pro.tile([128, 512], f32, name="pb", tag="pro")
                    nc.tensor.matmul(pb[:, 0:fw], rm_sb[:, v, :],
                                     rcs_sb[:, v, f0:f0 + fw],
                                     start=True, stop=True)
                    nc.scalar.activation(cs_sb[:, v, f0:f0 + fw], pb[:, 0:fw],
                                         AF.Copy)
                pbq = pro.tile([128, SQL], f32, name="pbq", tag="pro")
                nc.tensor.matmul(pbq[:, :], rm_sb[:, v, :], rcsq_sb[:, v, :],
                                 start=True, stop=True)
                nc.scalar.activation(csq_sb[:, v, :], pbq[:, :], AF.Copy)

            # halo mask broadcast [1,SQL] -> [128,SQL]
            pmh = pro.tile([128, SQL], f32, name="pmh", tag="pro")
            nc.tensor.matmul(pmh[:, :], onesB, hm_sb[:, :],
                             start=True, stop=True)
            nc.scalar.activation(maskB_sb[:, :], pmh[:, :], AF.Copy)

            def kproj(ch):
                for b in range(B):
                    for f0, fw in FCS:
                        pk = pro.tile([96, 512], f32, name="pk", tag="pro")
                        pks = pro.tile([96, 512], f32, name="pks", tag="pro")
                        for cc in range(2):
                            nc.tensor.matmul(
                                pk[:, 0:fw], wk_sb[:, 0, cc, ch, :],
                                kv_sb[:, b, cc, f0:f0 + fw],
                                start=(cc == 0), stop=(cc == 1))
                            nc.tensor.matmul(
                                pks[:, 0:fw], wk_sb[:, 1, cc, ch, :],
                                kv_sb[:, b, cc, f0:f0 + fw],
                                start=(cc == 0), stop=(cc == 1))
                        t1 = ptmp.tile([96, 512], bf16, name="t1", tag="ptmp")
                        t2 = ptmp.tile([96, 512], bf16, name="t2", tag="ptmp")
                        nc.vector.scalar_tensor_tensor(
                            t1[:, 0:fw], pk[:, 0:fw], qkb_sb[0:96, ch, 2:3],
                            cs_sb[0:96, 0, f0:f0 + fw], op0=OP.add,
                            op1=OP.mult)
                        nc.vector.scalar_tensor_tensor(
                            t2[:, 0:fw], pks[:, 0:fw], qkb_sb[0:96, ch, 3:4],
                            cs_sb[0:96, 1, f0:f0 + fw], op0=OP.add,
                            op1=OP.mult)
                        nc.gpsimd.tensor_add(kT_sb[0:96, b, ch, f0:f0 + fw],
                                             t1[:, 0:fw], t2[:, 0:fw])
                nh = NH[ch]
                for b in range(B):
                    sync.dma_start(kT_sb[16 * nh:17 * nh, b, ch, :],
                                   kTbias[ch, 0:nh, :])
                    sync.dma_start(kT_sb[17 * nh:17 * nh + 2, b, ch, :],
                                   kTbias[ch, 6:8, :])

            def qproj(ch):
                for b in range(B):
                    pq = pro.tile([96, SQL], f32, name="pq", tag="pro")
                    pqs = pro.tile([96, SQL], f32, name="pqs", tag="pro")
                    for cc in range(2):
                        nc.tensor.matmul(
                            pq[:, :], wq_sb[:, 0, cc, ch, :],
                            qsrc_sb[:, b, cc, :],
                            start=(cc == 0), stop=(cc == 1))
                        nc.tensor.matmul(
                            pqs[:, :], wq_sb[:, 1, cc, ch, :],
                            qsrc_sb[:, b, cc, :],
                            start=(cc == 0), stop=(cc == 1))
                    t1 = ptmp.tile([96, SQL], bf16, name="t1q", tag="ptmp")
                    t2 = ptmp.tile([96, SQL], bf16, name="t2q", tag="ptmp")
                    nc.vector.scalar_tensor_tensor(
                        t1[:, :], pq[:, :], qkb_sb[0:96, ch, 0:1],
                        csq_sb[0:96, 0, :], op0=OP.add, op1=OP.mult)
                    nc.vector.scalar_tensor_tensor(
                        t2[:, :], pqs[:, :], qkb_sb[0:96, ch, 1:2],
                        csq_sb[0:96, 1, :], op0=OP.add, op1=OP.mult)
                    nc.vector.tensor_add(qT_sb[0:96, b, ch, :],
                                         t1[:, :], t2[:, :])
                    for h in range(16):
                        if CHUNK_OF_HEAD[h] != ch:
                            continue
                        nc.vector.scalar_tensor_tensor(
                            qTv_sb[:, h, b, 0:SQL], qT_sb[:, b, ch, :],
                            pm_sb[:, h:h + 1], cq_sb[:, h, 0:SQL],
                            op0=OP.mult, op1=OP.add)

            def vproj(slots):
                for b in range(B):
                    for s in slots:
                        pv = pro.tile([128, 16, 16], f32, name="pv", tag="pro")
                        for cc in range(2):
                            nc.tensor.matmul(pv[:, :, :],
                                             kv_sb[:, b, cc, ts(s, 128)],
                                             wv_sb[:, cc, :],
                                             start=(cc == 0), stop=False)
                        nc.tensor.matmul(pv[:, :, :], kvo_sb[:, b, ts(s, 128)],
                                         wvb_sb[:, :], start=False, stop=True)
                        nc.vector.tensor_scalar(
                            vt_sb[:, b, s, :, 0:16], pv[:, :, :],
                            mv_sb[:, b, s:s + 1], None, op0=OP.mult)
                        nc.vector.tensor_scalar(
                            vt_sb[:, b, s, :, 32:33], onescol[:, :, :],
                            mv_sb[:, b, s:s + 1], None, op0=OP.mult)

            nc.gpsimd.memset(vt_sb[:, :, :, :, 16:32], 0.0)

            def attention(pi):
                h0 = 2 * pi
                ch = CHUNK_OF_HEAD[h0]
                n = N_P[pi]
                eal = peal.tile([128, 4, 2, SQP], bf16, name="eal", tag="eal")
                sync.dma_start(eal[:, :, :, :], eald[pi, :, :, :, :])
                for b in range(B):
                    po = po_pool.tile([33, 2, 512], f32, name="po", tag="po")
                    rhs_t = [None] * n

                    def emit_av(si):
                        for j in range(2):
                            nc.tensor.matmul(
                                po[:, j, 0:SQL],
                                vt_sb[:, b, si, h0 + j, :],
                                rhs_t[si][:, j, 0:SQL],
                                start=(si == 0), stop=(si == n - 1),
                                skip_group_check=True)

                    for si in range(n):
                        sc = pscore.tile([128, 2, 512], f32, name="sc",
                                         tag="sc")
                        for j in range(2):
                            nc.tensor.matmul(
                                sc[:, j, 0:SQL],
                                kT_sb[:, b, ch, ts(si, 128)],
                                qTv_sb[:, h0 + j, b, 0:SQL],
                                start=True, stop=True)
                        ex = pexp.tile([128, 2, SQP], bf16, name="ex",
                                       tag="ex")
                        nc.scalar.activation(ex[:, :, 0:SQL], sc[:, :, 0:SQL],
                                             AF.Exp)
                        if si < 4:
                            at = pattn.tile([128, 2, SQP], bf16, name="at",
                                            tag="at")
                            eng = nc.vector if (si + b) % 2 == 0 else nc.gpsimd
                            eng.tensor_mul(at[:, :, 0:SQL], ex[:, :, 0:SQL],
                                           eal[:, si, :, 0:SQL])
                            rhs_t[si] = at
                        else:
                            rhs_t[si] = ex
                        if si >= 1:
                            emit_av(si - 1)
                    emit_av(n - 1)
                    pidx = pi * B + b
                    nc.vector.tensor_copy(onum_sb[:, pidx, :, 0:SQL],
                                          po[:, :, 0:SQL])

            qproj(2); kproj(2); vproj(range(NSLOT))
            attention(6); attention(7)
            qproj(1); kproj(1)
            attention(5); attention(4); attention(3)
            qproj(0); kproj(0)
            attention(2); attention(1); attention(0)

        # ---- batched softmax division epilogue ----
        with ExitStack() as ph2b:
            pbc = ph2b.enter_context(
                tc.tile_pool(name="pbc", bufs=4, space="PSUM"))
            ptb = ph2b.enter_context(tc.tile_pool(name="ptb", bufs=1))
            den32 = ptb.tile([32, SQL], f32, name="den32", tag="lt")
            recf_sb = ptb.tile([1, 32, SQL], f32r, name="recf", tag="rf")
            # dens: onum row 32 -> partition-stacked [32, SQL]
            nc.sync.dma_start(denp_sb[:, :], onum_sb[32:33, :, :, 0:SQL])
            nc.vector.tensor_copy(den32[:, :], denp_sb[:, :])
            nc.vector.reciprocal_approx_fast(recp_sb[:, :], den32[:, :])
            recpr = ptb.tile([32, SQL], f32r, name="recpr", tag="lt2")
            nc.vector.tensor_copy(recpr[:, :], recp_sb[:, :])
            nc.sync.dma_start(recf_sb[:, :, :], recpr[:, :])
            for pi in range(8):
                for b in range(B):
                    pidx = pi * B + b
                    for j in range(2):
                        h = 2 * pi + j
                        rb = pbc.tile([32, 512], f32, name="rb", tag="rb")
                        nc.tensor.matmul(
                            rb[:, 0:SQL], ones32,
                            recf_sb[0:1, 2 * pidx + j, :],
                            start=True, stop=True)
                        ro = (h % 4) * 32
                        nc.vector.tensor_mul(
                            oall_sb[ro:ro + 32, b, h // 4, :],
                            onum_sb[0:32, pidx, j, 0:SQL], rb[:, 0:SQL])

        # ================= PHASE 3: conformer tail =================
        with ExitStack() as ph3:
            ptp = ph3.enter_context(
                tc.tile_pool(name="tail_psum", bufs=6, space="PSUM"))
            pt = ph3.enter_context(tc.tile_pool(name="tail_sb", bufs=20))
            pt1 = ph3.enter_context(tc.tile_pool(name="tail_sb1", bufs=12))

            def tv(pc, i):
                return tv_sb[:, pc, i:i + 1]

            def layernorm(xin, F, gi, bi, odt=f32r):
                """LN over channel dim (256 = partitions across 2 chunks)."""
                ps = ptp.tile([1, F], f32, name="ln_ps", tag="tp")
                ps2 = ptp.tile([1, F], f32, name="ln_ps2", tag="tp")
                for pc in range(2):
                    nc.tensor.matmul(ps[:, :], ones128, xin[pc][:, 0:F],
                                     start=(pc == 0), stop=(pc == 1))
                sqs = []
                for pc in range(2):
                    sq = pt1.tile([128, F], f32r, name="ln_sq", tag="pt1")
                    nc.scalar.activation(sq[:, :], xin[pc][:, 0:F], AF.Square)
                    sqs.append(sq)
                for pc in range(2):
                    nc.tensor.matmul(ps2[:, :], ones128, sqs[pc][:, :],
                                     start=(pc == 0), stop=(pc == 1))
                mean = pt1.tile([1, F], f32r, name="ln_mean", tag="pt1")
                nc.vector.tensor_scalar(mean[:, :], ps[:, :], 1.0 / D, None,
                                        op0=OP.mult)
                m2 = pt1.tile([1, F], f32, name="ln_m2", tag="pt1")
                nc.vector.tensor_mul(m2[:, :], mean[:, :], mean[:, :])
                var = pt1.tile([1, F], f32, name="ln_var", tag="pt1")
                nc.vector.scalar_tensor_tensor(
                    var[:, :], ps2[:, :], 1.0 / D, m2[:, :],
                    op0=OP.mult, op1=OP.subtract)
                sd = pt1.tile([1, F], f32, name="ln_sd", tag="pt1")
                nc.scalar.activation(sd[:, :], var[:, :], AF.Sqrt,
                                     bias=eps1[:, :])
                r32 = pt1.tile([1, F], f32, name="ln_r32", tag="pt1")
                nc.vector.reciprocal_approx_fast(r32[:, :], sd[:, :])
                rstd = pt1.tile([1, F], f32r, name="ln_rstd", tag="pt1")
                nc.vector.tensor_copy(rstd[:, :], r32[:, :])
                pmb = ptp.tile([128, F], f32, name="ln_pmb", tag="tp")
                nc.tensor.matmul(pmb[:, :], onesB, mean[:, :],
                                 start=True, stop=True)
                prb = ptp.tile([128, F], f32, name="ln_prb", tag="tp")
                nc.tensor.matmul(prb[:, :], onesB, rstd[:, :],
                                 start=True, stop=True)
                outs = []
                for pc in range(2):
                    t = pt.tile([128, F], f32, name="ln_t", tag="pt")
                    nc.vector.tensor_sub(t[:, :], xin[pc][:, 0:F], pmb[:, :])
                    t2 = pt.tile([128, F], f32, name="ln_t2", tag="pt")
                    nc.vector.tensor_mul(t2[:, :], t[:, :], prb[:, :])
                    o = pt.tile([128, F], odt, name="ln_o", tag="pt")
                    nc.vector.tensor_scalar(o[:, :], t2[:, :], tv(pc, gi),
                                            tv(pc, bi), op0=OP.mult, op1=OP.add)
                    outs.append(o)
                return outs

            def mm4(widx, rhs_tiles, F, name):
                outs = []
                for pc in range(2):
                    p = ptp.tile([128, F], f32, name=name, tag="tp")
                    for cc in range(2):
                        nc.tensor.matmul(p[:, :],
                                         tw_sb[:, widx, cc, ts(pc, 128)],
                                         rhs_tiles[cc][:, 0:F],
                                         start=(cc == 0), stop=(cc == 1))
                    outs.append(p)
                return outs

            # o-projection + residual (per b)
            x1 = {}
            for b in range(B):
                oproj = []
                for pc in range(2):
                    px = ptp.tile([128, SQL], f32, name="px", tag="tp")
                    for c4 in range(4):
                        nc.tensor.matmul(
                            px[:, :], wop_sb[:, c4, ts(pc, 128)],
                            oall_sb[:, b, c4, :],
                            start=(c4 == 0), stop=(c4 == 3))
                    x1p = pt.tile([128, SQL], f32r, name="x1p", tag="pt")
                    nc.vector.scalar_tensor_tensor(
                        x1p[:, :], px[:, :], tv(pc, 0),
                        qsrcf_sb[:, b, pc, :], op0=OP.add, op1=OP.add)
                    oproj.append(x1p)
                x1[b] = layernorm(oproj, SQL, 1, 2)
            c0 = {b: layernorm(x1[b], SQL, 3, 4) for b in range(B)}
            # pw1 + gelu + halo mask
            cm = {}
            for b in range(B):
                cp = mm4(1, c0[b], SQL, "pc1")
                cmb = []
                for pc in range(2):
                    cg = pt.tile([128, SQL], f32, name="cg", tag="pt")
                    nc.scalar.activation(cg[:, :], cp[pc][:, :], AF.Gelu,
                                         bias=tv(pc, 5))
                    cmt = pt.tile([128, SQL], f32r, name="cmt", tag="pt")
                    nc.gpsimd.tensor_mul(cmt[:, :], cg[:, :], maskB_sb[:, :])
                    cmb.append(cmt)
                cm[b] = cmb
            # depthwise conv (3 taps) + BN + hardswish
            hsw = {}
            for b in range(B):
                hswb = []
                for pc in range(2):
                    cmp_ = cm[b][pc]
                    a1 = pt.tile([128, WIN], f32, name="a1", tag="pt")
                    nc.vector.tensor_scalar(a1[:, :], cmp_[:, 1:WIN + 1],
                                            tv(pc, 7), None, op0=OP.mult)
                    a2 = pt.tile([128, WIN], f32, name="a2", tag="pt")
                    nc.vector.scalar_tensor_tensor(
                        a2[:, :], cmp_[:, 0:WIN], tv(pc, 6), a1[:, :],
                        op0=OP.mult, op1=OP.add)
                    a3 = pt.tile([128, WIN], f32, name="a3", tag="pt")
                    nc.vector.scalar_tensor_tensor(
                        a3[:, :], cmp_[:, 2:WIN + 2], tv(pc, 8), a2[:, :],
                        op0=OP.mult, op1=OP.add)
                    bn = pt.tile([128, WIN], f32, name="bn", tag="pt")
                    nc.scalar.activation(bn[:, :], a3[:, :], AF.Identity,
                                         bias=tv(pc, 10), scale=tv(pc, 9))
                    h1 = pt.tile([128, WIN], f32, name="h1", tag="pt")
                    nc.vector.tensor_scalar(h1[:, :], bn[:, :], 3.0, 6.0,
                                            op0=OP.add, op1=OP.min)
                    h2 = pt.tile([128, WIN], f32, name="h2", tag="pt")
                    nc.vector.tensor_scalar(h2[:, :], h1[:, :], 0.0, None,
                                            op0=OP.max)
                    hst = pt.tile([128, WIN], f32r, name="hst", tag="pt")
                    nc.vector.scalar_tensor_tensor(
                        hst[:, :], bn[:, :], 1.0 / 6.0, h2[:, :],
                        op0=OP.mult, op1=OP.mult)
                    hswb.append(hst)
                hsw[b] = hswb
            # pw2
            x2 = {}
            for b in range(B):
                p2 = mm4(2, hsw[b], WIN, "p2")
                x2b = []
                for pc in range(2):
                    x2t = pt.tile([128, WIN], f32r, name="x2t", tag="pt")
                    nc.vector.tensor_scalar(x2t[:, :], p2[pc][:, :],
                                            tv(pc, 11), None, op0=OP.add)
                    x2b.append(x2t)
                x2[b] = x2b
            # FFN
            gg = {}
            for b in range(B):
                p3 = mm4(3, x2[b], WIN, "p3")
                ggb = []
                for pc in range(2):
                    g1 = pt.tile([128, WIN], f32r, name="g1", tag="pt")
                    nc.scalar.activation(g1[:, :], p3[pc][:, :], AF.Gelu,
                                         bias=tv(pc, 12))
                    ggb.append(g1)
                gg[b] = ggb
            for b in range(B):
                p4 = mm4(4, gg[b], WIN, "p4")
                x3 = []
                for pc in range(2):
                    x3t = pt.tile([128, WIN], f32r, name="x3t", tag="pt")
                    nc.vector.scalar_tensor_tensor(
                        x3t[:, :], p4[pc][:, :], tv(pc, 13), x2[b][pc][:, :],
                        op0=OP.add, op1=OP.add)
                    x3.append(x3t)
                xo = layernorm(x3, WIN, 14, 15, odt=f32)
                for pc in range(2):
                    sync.dma_start(out[b, ts(pc, 128), :], xo[pc][:, :])

        for _t, _free in reversed(_keep):
            _free()

    nc.compile()
    return nc


# ---------------------------------------------------------------------------
# Host-side input prep (sharding)
# ---------------------------------------------------------------------------

def host_prep(inputs):
    f32 = np.float32

    src = np.asarray(inputs["src"], f32)
    alibi = np.asarray(inputs["alibi_bias"], f32)
    pos_emb = np.asarray(inputs["pos_emb"], f32)
    mask = np.asarray(inputs["mask"])

    slopes = -alibi[:, 0, 1].astype(np.float64)  # alibi[h,0,1] = -slope_h
    exp_slopes = 2.0 ** (-8.0 * (np.arange(H) + 1) / H)
    assert np.allclose(slopes, exp_slopes, rtol=1e-3), "unexpected alibi slopes"

    cos = np.cos(pos_emb).astype(f32)  # [S, 8]
    sin = np.sin(pos_emb).astype(f32)

    # swap perm within each head: j -> (j+8)%16
    jj = np.arange(D)
    swap = (jj // HD) * HD + (jj % HD + HD // 2) % HD

    wq, wk, wvm = [np.asarray(inputs[k], f32) for k in ("wq", "wk", "wv")]
    bq, bk, bv = [np.asarray(inputs[k], f32) for k in ("bq", "bk", "bv")]

    # chunked weight cols (chunk ch = global d cols 96ch : 96ch+16*NH[ch])
    def chunk_cols(m):  # m [D, D] -> [D, 3, 96]
        outm = np.zeros((D, 3, 96), f32)
        for ch in range(3):
            w = 16 * NH[ch]
            outm[:, ch, 0:w] = m[:, 96 * ch:96 * ch + w]
        return outm

    wq2 = np.stack([chunk_cols(wq.T), chunk_cols(wq.T[:, swap])]).astype(BF16)
    wk2 = np.stack([chunk_cols(wk.T), chunk_cols(wk.T[:, swap])]).astype(BF16)
    wv2 = np.concatenate([wvm.T, bv[None, :]], 0).astype(BF16)

    # qkb [128, 3, 4]: chunk-local rows
    qkb = np.zeros((128, 3, 4), f32)
    for ch in range(3):
        w = 16 * NH[ch]
        sl = slice(96 * ch, 96 * ch + w)
        qkb[0:w, ch, 0] = bq[sl]
        qkb[0:w, ch, 1] = bq[swap][sl]
        qkb[0:w, ch, 2] = bk[sl]
        qkb[0:w, ch, 3] = bk[swap][sl]

    # Mcos[i, v, r]: cos-select (r%8==i); Msin adds sign by half
    r = np.arange(128)
    mc = (r[None, :] % 8 == np.arange(8)[:, None]).astype(f32)
    sgn_r = np.where((r % HD) < HD // 2, -1.0, 1.0).astype(f32)
    ropem = np.ascontiguousarray(
        np.stack([mc, mc * sgn_r[None, :]], 1)).astype(BF16)

    # head-slot row masks [128, 16]
    pm = np.zeros((128, 16), f32)
    for h in range(16):
        ch = CHUNK_OF_HEAD[h]
        i = h - HBASE[ch]
        pm[16 * i:16 * i + 16, h] = 1.0

    # q-side constant slot tiles [128, 16, SQP]
    fidx = np.arange(SQL, dtype=f32)
    cqt = np.zeros((128, 16, SQP), f32)
    for h in range(16):
        ch = CHUNK_OF_HEAD[h]
        nh = NH[ch]
        i = h - HBASE[ch]
        sl_f = np.float32(slopes[h])
        cqt[16 * nh + i, h, 0:SQL] = 1.0
        cqt[17 * nh, h, 0:SQL] = (-sl_f * fidx).astype(BF16).astype(f32)
        cqt[17 * nh + 1, h, 0:SQL] = (-sl_f * (257.0 - fidx)
                                      ).astype(BF16).astype(f32)
    cqt = cqt.astype(BF16)

    # tail weights / vectors
    wo, pw1, pw2, w1m, w2m = [np.asarray(inputs[k], f32)
                              for k in ("wo", "pw1_w", "pw2_w", "w1", "w2")]
    tailw = np.ascontiguousarray(
        np.stack([wo.T, pw1.T, pw2.T, w1m.T, w2m.T])).astype(f32)
    dww = np.asarray(inputs["dw_w"], f32)  # [D, 1, 3]
    sbn = (np.asarray(inputs["bn_g"], f32) /
           np.sqrt(np.asarray(inputs["bn_var"], f32) + 1e-5))
    tbn = ((np.asarray(inputs["dw_b"], f32) -
            np.asarray(inputs["bn_mean"], f32)) * sbn +
           np.asarray(inputs["bn_b"], f32))
    vecs = [inputs["bo"], inputs["n1_g"], inputs["n1_b"], inputs["ln_g"],
            inputs["ln_b"], inputs["pw1_b"], dww[:, 0, 0], dww[:, 0, 1],
            dww[:, 0, 2], sbn, tbn, inputs["pw2_b"], inputs["b1"],
            inputs["b2"], inputs["n2_g"], inputs["n2_b"]]
    tailv = np.stack([np.asarray(v, f32) for v in vecs], -1)  # [D, 16]
    tailv = np.ascontiguousarray(
        tailv.reshape(2, 128, 16).transpose(1, 0, 2)).astype(f32)

    wop = np.zeros((512, D), f32)
    r512 = np.arange(512)
    real = (r512 % 32) < 16
    dsrc = (r512 // 32) * 16 + (r512 % 32)
    wop[real, :] = wo.T[dsrc[real], :]

    srckv_g = np.concatenate(
        [src.transpose(0, 2, 1), np.ones((B, 1, S), f32)], 1)  # [B, 257, S]
    maskvec = (1.0 - mask.astype(f32))  # [B, S]
    p128 = np.arange(128)

    in_maps = []
    for c in range(NCORES):
        q0 = c * WIN - 1
        wb = 2 * c - 1
        diag = [wb, wb + 1, wb + 2, wb + 3]

        def mindist(g):
            return max(q0 - (128 * g + 127), 128 * g - (q0 + 257), 0)

        rest = sorted((g for g in range(NB) if g not in diag), key=mindist)
        perm = [(g if 0 <= g < NB else -1) for g in diag] + rest
        perm = perm + [-1] * (NSLOT - len(perm))

        # permuted / padded per-core tensors
        srckv_c = np.zeros((B, D + 1, SK), f32)
        maskv_c = np.zeros((128, B, NSLOT), f32)
        ropecs_c = np.zeros((8, 2, SK), f32)
        for s, g in enumerate(perm):
            if g < 0:
                continue
            sl = slice(128 * s, 128 * s + 128)
            gsl = slice(128 * g, 128 * g + 128)
            srckv_c[:, :, sl] = srckv_g[:, :, gsl]
            maskv_c[:, :, s] = maskvec[:, gsl].T
            ropecs_c[:, 0, sl] = cos[gsl, :].T
            ropecs_c[:, 1, sl] = sin[gsl, :].T

        # kT bias rows [3, 8, SK]
        kTb = np.zeros((3, 8, SK), f32)
        skpos = np.zeros(SK, np.int64)
        isdum = np.zeros(SK, bool)
        for s, g in enumerate(perm):
            sl = slice(128 * s, 128 * s + 128)
            if g < 0:
                isdum[sl] = True
            else:
                skpos[sl] = 128 * g + p128
        edged = np.maximum(np.maximum(q0 - skpos, skpos - (q0 + 257)), 0
                           ).astype(f32)
        indL = ((skpos < q0) & ~isdum).astype(f32)
        indR = ((skpos > q0 + 257) & ~isdum).astype(f32)
        for ch in range(3):
            for i in range(NH[ch]):
                h = HBASE[ch] + i
                bias_row = (-np.float32(slopes[h]) * edged)
                bias_row[isdum] = -30.0
                kTb[ch, i, :] = bias_row
            kTb[ch, 6, :] = indL
            kTb[ch, 7, :] = indR
        kTb_bf = kTb.astype(BF16)

        # diag exp(alibi) compensated for the matmul bias terms
        eald_c = np.ones((8, 128, 4, 2, SQP), f32)
        qpos = np.clip(q0 + np.arange(SQL), 0, S - 1)
        for pi in range(8):
            for si in range(4):
                g = perm[si]
                if g < 0:
                    continue
                sk = 128 * g + p128
                for j in range(2):
                    h = 2 * pi + j
                    ch = CHUNK_OF_HEAD[h]
                    i = h - HBASE[ch]
                    b_bf = kTb_bf[ch, i, 128 * si:128 * si + 128].astype(f32)
                    colL = cqt[17 * NH[ch], h, 0:SQL].astype(f32)
                    colR = cqt[17 * NH[ch] + 1, h, 0:SQL].astype(f32)
                    bias2d = (b_bf[:, None]
                              + indL[128 * si:128 * si + 128][:, None]
                              * colL[None, :]
                              + indR[128 * si:128 * si + 128][:, None]
                              * colR[None, :])
                    a = alibi[h][np.ix_(sk, qpos)]
                    eald_c[pi, :, si, j, 0:SQL] = np.exp(
                        np.minimum(a - bias2d, 0.0))

        # q-window slice with zero padding
        sq = np.zeros((B, SQL, D), f32)
        lo, hi = max(q0, 0), min(q0 + SQL, S)
        sq[:, lo - q0:hi - q0, :] = src[:, lo:hi, :]
        srcqT = sq.transpose(0, 2, 1)  # [B, D, SQL]

        csq = np.zeros((8, 2, SQL), f32)
        csq[:, 0, lo - q0:hi - q0] = 0.25 * cos[lo:hi, :].T
        csq[:, 1, lo - q0:hi - q0] = 0.25 * sin[lo:hi, :].T

        halo = np.ones((1, SQL), f32)
        if q0 < 0:
            halo[0, 0] = 0.0
        if q0 + SQL > S:
            halo[0, SQL - 1] = 0.0

        in_maps.append({
            "srckv": srckv_c.astype(BF16),
            "srcqbf": srcqT.astype(BF16),
            "srcqf": srcqT.astype(f32),
            "wq2": wq2, "wk2": wk2, "wv": wv2,
            "ropecs": ropecs_c.astype(BF16),
            "ropecsq": csq.astype(BF16),
            "ropem": ropem,
            "qkb": qkb,
            "maskv": maskv_c,
            "kTbias": kTb_bf,
            "cq": cqt,
            "eald": eald_c.astype(BF16),
            "tailw": tailw,
            "tailv": tailv,
            "halom": halo,
            "pmask": pm,
            "wop": wop,
            "onesr": np.ones((128, 128), f32),
        })
    return in_maps


def kernel(**inputs) -> np.ndarray:
    _ensure_ntff_hook()
    from concourse.bass_utils import run_bass_kernel_spmd

    if "nc" not in _COMPILED:
        _COMPILED["nc"] = build_nc()
    nc = _COMPILED["nc"]

    in_maps = host_prep(inputs)
    trace = os.environ.get("KERNEL_TRACE", "0") == "1"
    res = run_bass_kernel_spmd(nc, in_maps, core_ids=list(range(NCORES)),
                               trace=trace)
    kernel.last_result = res

    b_, s_, d_ = inputs["src"].shape
    full = np.empty((b_, s_, d_), np.float32)
    for c in range(NCORES):
        o = res.results[c]["out"]  # [B, D, win]
        full[:, c * WIN:(c + 1) * WIN, :] = o.transpose(0, 2, 1)
    return full



# revision 27
# speedup vs baseline: 1.2114x; 1.2114x over previous
"""Trainium2 Bass kernel for ALiBiConformerEncoderLayer (8-core SPMD).

Sharding: sequence windows (256 queries + 1 halo col each side) per core.

v2 redesign vs the 311us baseline:
- No big memsets: matmul contraction restricted to K=104/70 rows (weights
  zero-padded to 104 cols), vt packed per-slot with static ones column.
- vproj bias matmul dropped (bo' = bo + wo@bv host fold); per-slot head
  restriction (only heads whose pair reaches that slot).
- kproj column restriction per chunk (512 / 2048 / 2176).
- Inclusive indicator rows (sk<=q0, sk>=q0+257) make the factorized alibi
  exact on the window-boundary rows: only 2 diag slots need the 2D
  exp(alibi) multiply (eald halved).
- AV matmuls land 4 pairs per PSUM tile via tile_position col offsets;
  division epilogue = 8 PSUM->SBUF copies + tiny den DMAs + 4-row
  indicator matmul broadcast + 8 muls (was 16 copies + 32 bcasts + 32 muls).
- b-split attention: all pairs b0, then b1; b0's oproj/LN1/LN2 overlap b1's
  attention. Act-table discipline: only {exp, square, identity, copy, relu}
  until the single gelu-set switch (greedy table loader thrashes otherwise).
- LN rsqrt via DVE Newton iteration (bit-trick seed) - no Ln/Sqrt tables.
"""
import os
import sys
import types
from contextlib import ExitStack

import numpy as np
import ml_dtypes

BF16 = ml_dtypes.bfloat16

# Problem constants (hardcoded; kernel.py must be self-contained)
B, S, D, H, HD = 2, 2048, 256, 16, 16
NCORES = 8
WIN = S // NCORES          # 256
SQL = WIN + 2              # query cols incl 1 halo each side
SQP = 260                  # 4-elem aligned bf16 stride
NB = S // 128              # 16 real sk blocks
NSLOT = 17                 # 16 real + 1 dummy (window-centered permutation)
SK = NSLOT * 128

# kT chunking: heads per chunk, base head of chunk
NH = [6, 6, 4]
HBASE = [0, 6, 12]
CHUNK_OF_HEAD = [0] * 6 + [1] * 6 + [2] * 4
KCH = [104, 104, 70]       # contraction rows per chunk (vals+bias+ind)
BROW = [96, 96, 64]        # first bias row per chunk (16*nh)
QVR_N = [8, 8, 6]          # qTv bias rows per head
KCOLS = [512, 2048, 2176]  # kT cols needed per chunk

# Block-skip: pair pi covers heads (2pi, 2pi+1); reach D = T/slope(2pi+1).
N_P = [4, 4, 4, 6, 10, 16, 17, 17]

# vt packing: first head needing slot s, offsets
HLO = [0] * 4 + [6] * 2 + [8] * 4 + [10] * 6 + [12]
VOFF = [0] * NSLOT
for _s in range(1, NSLOT):
    VOFF[_s] = VOFF[_s - 1] + (16 - HLO[_s - 1])
NVT = VOFF[-1] + (16 - HLO[-1])  # 156

FCS_CH = [
    [(0, 512)],
    [(0, 512), (512, 512), (1024, 512), (1536, 512)],
    [(0, 512), (512, 512), (1024, 512), (1536, 512), (2048, 128)],
]

_COMPILED = {}


def _ensure_ntff_hook():
    """Install the axon NTFF profiling hook if the image lacks antenv.axon_hooks."""
    try:
        import antenv.axon_hooks  # noqa: F401
        return
    except ImportError:
        pass
    try:
        from trn_agent_boot.trn_boot import _ntff_profile_via_ctypes
        hook = _ntff_profile_via_ctypes('/opt/axon/libaxon_pjrt.so')
    except Exception:
        hook = None
    mod = types.ModuleType('antenv.axon_hooks')
    mod.get_axon_ntff_profile_hook = lambda: hook
    mod.set_axon_ntff_profile_hook = lambda h: None
    sys.modules['antenv.axon_hooks'] = mod


# ---------------------------------------------------------------------------
# Graph builder
# ---------------------------------------------------------------------------

def build_nc():
    import concourse.bass as bass  # noqa: F401
    import concourse.tile as tile
    from concourse import bacc, mybir
    from concourse.bass import ts

    f32 = mybir.dt.float32
    f32r = mybir.dt.float32r
    bf16 = mybir.dt.bfloat16
    i32 = mybir.dt.int32
    AF = mybir.ActivationFunctionType
    OP = mybir.AluOpType

    nc = bacc.Bacc(None, target_bir_lowering=False)

    P = {}
    def dram(name, shape, dt):
        P[name] = nc.declare_dram_parameter(name, list(shape), dt, isOutput=False)
        return P[name]

    srckv = dram("srckv", [B, D, SK], bf16)         # permuted src.T
    srcq = dram("srcq", [B, D, SQL], bf16)
    wq2 = dram("wq2", [2, D, 3, 104], bf16)         # wqT cols + swapped, chunked
    wk2 = dram("wk2", [2, D, 3, 104], bf16)
    wv = dram("wv", [D, D], bf16)                   # wvT (bias folded into bo')
    ropecs = dram("ropecs", [8, 2, SK], bf16)       # cosT, sinT (k side, permuted)
    ropecsq = dram("ropecsq", [8, 2, SQL], bf16)    # 0.25*cos/sin (q side)
    ropem = dram("ropem", [8, 2, 128], bf16)        # Mcos, Msin(signed)
    qkb = dram("qkb", [128, 3, 4], f32)             # bq, bqs, bk, bks chunk rows
    kTbias = dram("kTbias", [B, 3, 8, SK], bf16)    # B rows (0:nh) + ind (6:8)
    qvr = dram("qvr", [16, 8, SQL], bf16)           # qTv bias rows per head
    eald = dram("eald", [8, 128, 2, 2, SQP], bf16)  # diag exp(alibi), slots 1,2
    tailw = dram("tailw", [5, D, D], f32r)          # woT pw1T pw2T w1T w2T
    tailv = dram("tailv", [128, 2, 16], f32)
    halom = dram("halom", [1, SQL], bf16)
    pmask = dram("pmask", [128, 16], f32)           # head-slot row masks
    onesbf = dram("onesbf", [128, 128], bf16)
    onesr = dram("onesr", [128, 128], f32r)
    wop = dram("wop", [2, 2, 128, D], bf16)         # woT rows per (grp, j)
    ind4 = dram("ind4", [4, 128], f32r)             # recB indicator rows
    sel4 = dram("sel4", [128, 4], bf16)             # den-row selector
    out = nc.declare_dram_parameter("out", [B, D, WIN], f32, isOutput=True)
    KDBG = os.environ.get("KDBG", "0") == "1"
    if KDBG:
        d_onum = nc.declare_dram_parameter("d_onum", [128, B, 2, 2, SQP], bf16,
                                           isOutput=True)
        d_oall = nc.declare_dram_parameter("d_oall", [128, B, 2, 2, SQP], bf16,
                                           isOutput=True)
        d_den = nc.declare_dram_parameter("d_den", [36, B, 2, SQP], bf16,
                                          isOutput=True)
        d_rec = nc.declare_dram_parameter("d_rec", [36, B, 2, SQP], f32,
                                          isOutput=True)
        d_qtv = nc.declare_dram_parameter("d_qtv", [128, 16, B, SQP], bf16,
                                          isOutput=True)
        d_kt0 = nc.declare_dram_parameter("d_kt0", [128, B, 512], bf16,
                                          isOutput=True)
        d_vt = nc.declare_dram_parameter("d_vt", [128, B, NVT, 20], bf16,
                                         isOutput=True)
        d_eal = nc.declare_dram_parameter("d_eal", [128, 8, 2, 2, SQP], bf16,
                                          isOutput=True)
        d_sc = nc.declare_dram_parameter("d_sc", [128, 3, 2, SQP], f32,
                                         isOutput=True)

    with ExitStack() as top, tile.TileContext(nc) as tc:
        _keep = []
        def ctile(shape, dt, name):
            t, _free = tc.tile(list(shape), dt, name=name)
            _keep.append((t, _free))
            return t

        sync = nc.sync

        # ---- persistent SBUF ----
        tw_sb = ctile([128, 5, 2, D], f32r, "tw_sb")
        tv_sb = ctile([128, 2, 16], f32, "tv_sb")
        qkb_sb = ctile([128, 3, 4], f32, "qkb_sb")
        pm_sb = ctile([128, 16], f32, "pm_sb")
        onesbf_sb = ctile([128, 128], bf16, "onesbf_sb")
        onesr_sb = ctile([128, 128], f32r, "onesr_sb")
        ind4_sb = ctile([4, 128], f32r, "ind4_sb")
        sel4_sb = ctile([128, 4], bf16, "sel4_sb")
        wop_sb = ctile([128, 2, 2, D], bf16, "wop_sb")
        qsrc_sb = ctile([128, B, 2, SQL], bf16, "qsrc_sb")
        kT0 = ctile([128, B, 512], bf16, "kT0")
        kT1 = ctile([128, B, 2048], bf16, "kT1")
        kT2 = ctile([128, B, SK], bf16, "kT2")
        KTT = [kT0, kT1, kT2]
        qTv_sb = ctile([128, 16, B, SQP], bf16, "qTv_sb")
        vt_sb = ctile([128, B, NVT, 20], bf16, "vt_sb")
        eal_sb = ctile([128, 8, 2, 2, SQP], bf16, "eal_sb")
        onum_sb = ctile([128, B, 2, 2, SQP], bf16, "onum_sb")
        oall_sb = ctile([128, B, 2, 2, SQP], bf16, "oall_sb")
        denf4 = ctile([4, B, 2, 2, SQP], f32, "denf4")
        recf4 = ctile([4, B, 2, 2, SQP], f32, "recf4")
        rec4 = ctile([4, B, 2, 2, SQP], f32r, "rec4")
        maskB_sb = ctile([128, SQL], bf16, "maskB_sb")
        eps1 = ctile([1, 2], f32, "eps1")
        dsc_t = ctile([128, 3, 2, SQP], f32, "dsc_t") if KDBG else None

        nc.vector.memset(eps1[:, :], 1e-5)
        # Act warmup: force the exp table load at t=0 (first Act op)
        nc.scalar.activation(eps1[:, 1:2], eps1[:, 0:1], AF.Exp)

        # ---- initial DMA: priority order ----
        # prologue-only SBUF (weights, rope tables, kv): freed before the
        # tail pool opens so left/right SBUF stacks never collide.
        rm_es = ExitStack()
        p1c = rm_es.enter_context(tc.tile_pool(name="p1c", bufs=1))
        def p1tile(shape, dt, name):
            return p1c.tile(list(shape), dt, name=name, tag=name)

        wq_sb = p1tile([128, 2, 2, 3, 104], bf16, "wq_sb")
        wk_sb = p1tile([128, 2, 2, 3, 104], bf16, "wk_sb")
        wv_sb = p1tile([128, 2, D], bf16, "wv_sb")
        rm_sb = p1tile([8, 2, 128], bf16, "rm_sb")
        rcs_sb = p1tile([8, 2, SK], bf16, "rcs_sb")
        rcsq_sb = p1tile([8, 2, SQL], bf16, "rcsq_sb")
        kv_sb = p1tile([128, B, 2, SK], bf16, "kv_sb")
        cs_sb = p1tile([128, 2, SK], bf16, "cs_sb")
        csq_sb = p1tile([128, 2, SQL], bf16, "csq_sb")
        qT_sb = p1tile([128, B, 3, SQL], bf16, "qT_sb")
        hm_sb = p1tile([1, SQL], bf16, "hm_sb")

        sync.dma_start(rm_sb[:, :, :], ropem[:, :, :])
        sync.dma_start(rcsq_sb[:, :, :], ropecsq[:, :, :])
        sync.dma_start(rcs_sb[:, :, :], ropecs[:, :, :])
        sync.dma_start(qkb_sb[:, :, :], qkb[:, :, :])
        sync.dma_start(pm_sb[:, :], pmask[:, :])
        sync.dma_start(onesbf_sb[:, :], onesbf[:, :])
        sync.dma_start(onesr_sb[:, :], onesr[:, :])
        for b in range(B):
            for cc in range(2):
                sync.dma_start(qsrc_sb[:, b, cc, :], srcq[b, ts(cc, 128), :])
        for v in range(2):
            for cc in range(2):
                sync.dma_start(wq_sb[:, v, cc, :, :], wq2[v, ts(cc, 128), :, :])
                sync.dma_start(wk_sb[:, v, cc, :, :], wk2[v, ts(cc, 128), :, :])
        # srckv: diag slots (cols 0:512) first so vproj/attn(ch0) start early
        for b in range(B):
            for cc in range(2):
                sync.dma_start(kv_sb[:, b, cc, 0:512],
                               srckv[b, ts(cc, 128), 0:512])
        for cc in range(2):
            sync.dma_start(wv_sb[:, cc, :], wv[ts(cc, 128), :])
        for pi in range(3):
            sync.dma_start(eal_sb[:, pi, :, :, :], eald[pi, :, :, :, :])
        for b in range(B):
            for cc in range(2):
                sync.dma_start(kv_sb[:, b, cc, 512:SK],
                               srckv[b, ts(cc, 128), 512:SK])
        for pi in range(3, 8):
            sync.dma_start(eal_sb[:, pi, :, :, :], eald[pi, :, :, :, :])
        sync.dma_start(ind4_sb[:, :], ind4[:, :])
        sync.dma_start(sel4_sb[:, :], sel4[:, :])
        for g in range(2):
            for j in range(2):
                sync.dma_start(wop_sb[:, g, j, :], wop[g, j, :, :])
        sync.dma_start(hm_sb[:, :], halom[:, :])
        sync.dma_start(tv_sb[:, :, :], tailv[:, :, :])
        for w in range(5):
            for cc in range(2):
                sync.dma_start(tw_sb[:, w, cc, :], tailw[w, ts(cc, 128), :])

        # static ones column of vt (AV denominator row); dummy slots still
        # contribute only exp(-30)~1e-13 to the denominator.
        nc.gpsimd.memset(vt_sb[:, :, :, 16:17], 1.0)

        ones128r = onesr_sb[:, 0:1]
        onesBr = onesr_sb[0:1, :]
        ones128b = onesbf_sb[:, 0:1]
        onesBb = onesbf_sb[0:1, :]

        def tv(pc, i):
            return tv_sb[:, pc, i:i + 1]

        # ================= PHASE 1: prologue + attention =================
        es_pro = ExitStack()
        es_attn = ExitStack()
        pro = es_pro.enter_context(
            tc.tile_pool(name="pro_psum", bufs=2, space="PSUM", side="right"))
        ptmp = es_attn.enter_context(
            tc.tile_pool(name="pro_tmp", bufs=6, side="right"))
        pscore = es_attn.enter_context(
            tc.tile_pool(name="pscore", bufs=2, space="PSUM"))
        ppo = es_attn.enter_context(
            tc.tile_pool(name="ppo", bufs=1, space="PSUM"))
        pexp = es_attn.enter_context(
            tc.tile_pool(name="pexp", bufs=3, side="right"))
        pattn = es_attn.enter_context(
            tc.tile_pool(name="pattn", bufs=2, side="right"))

        po_t = ppo.tile([128, 2, 512], f32, name="po_t", tag="po")
        # zero once: AV writes only 17-row bands; copies read all 128 rows.
        nc.vector.memset(po_t[:, :, :], 0.0)

        # rope broadcast tiles: cs[r, t] = M[., r] rows x cosT/sinT
        def rope_q():
            for v in range(2):
                pbq = pro.tile([128, SQL], f32, name="pbq", tag="pro")
                nc.tensor.matmul(pbq[:, :], rm_sb[:, v, :], rcsq_sb[:, v, :],
                                 start=True, stop=True)
                nc.scalar.activation(csq_sb[:, v, :], pbq[:, :], AF.Copy)

        def rope_k(f0, fw):
            for v in range(2):
                pb = pro.tile([128, 512], f32, name="pb", tag="pro")
                nc.tensor.matmul(pb[:, 0:fw], rm_sb[:, v, :],
                                 rcs_sb[:, v, f0:f0 + fw],
                                 start=True, stop=True)
                nc.scalar.activation(cs_sb[:, v, f0:f0 + fw], pb[:, 0:fw],
                                     AF.Copy)

        def qproj(ch):
            nh = NH[ch]
            te = 96 if ch < 2 else 64   # ts write extent (ch2 keeps 64:70 free)
            for b in range(B):
                pq = pro.tile([96, SQL], f32, name="pq", tag="pro")
                pqs = pro.tile([96, SQL], f32, name="pqs", tag="pro")
                for cc in range(2):
                    nc.tensor.matmul(
                        pq[:, :], wq_sb[:, 0, cc, ch, 0:96],
                        qsrc_sb[:, b, cc, :], start=(cc == 0), stop=(cc == 1))
                    nc.tensor.matmul(
                        pqs[:, :], wq_sb[:, 1, cc, ch, 0:96],
                        qsrc_sb[:, b, cc, :], start=(cc == 0), stop=(cc == 1))
                t1 = ptmp.tile([96, SQL], bf16, name="t1q", tag="ptmp")
                t2 = ptmp.tile([96, SQL], bf16, name="t2q", tag="ptmp")
                nc.vector.scalar_tensor_tensor(
                    t1[:, :], pq[:, :], qkb_sb[0:96, ch, 0:1],
                    csq_sb[0:96, 0, :], op0=OP.add, op1=OP.mult)
                nc.vector.scalar_tensor_tensor(
                    t2[:, :], pqs[:, :], qkb_sb[0:96, ch, 1:2],
                    csq_sb[0:96, 1, :], op0=OP.add, op1=OP.mult)
                nc.gpsimd.tensor_add(qT_sb[0:96, b, ch, :], t1[:, :], t2[:, :])
                for i in range(nh):
                    h = HBASE[ch] + i
                    nc.vector.tensor_scalar(
                        qTv_sb[0:te, h, b, 0:SQL], qT_sb[0:te, b, ch, :],
                        pm_sb[0:te, h:h + 1], None, op0=OP.mult)
                    sync.dma_start(
                        qTv_sb[BROW[ch]:BROW[ch] + QVR_N[ch], h, b, 0:SQL],
                        qvr[h, 0:QVR_N[ch], :])

        def kproj(ch):
            nh = NH[ch]
            kt = KTT[ch]
            for b in range(B):
                for f0, fw in FCS_CH[ch]:
                    pk = pro.tile([104, 512], f32, name="pk", tag="pro")
                    pks = pro.tile([104, 512], f32, name="pks", tag="pro")
                    for cc in range(2):
                        nc.tensor.matmul(
                            pk[:, 0:fw], wk_sb[:, 0, cc, ch, :],
                            kv_sb[:, b, cc, f0:f0 + fw],
                            start=(cc == 0), stop=(cc == 1))
                        nc.tensor.matmul(
                            pks[:, 0:fw], wk_sb[:, 1, cc, ch, :],
                            kv_sb[:, b, cc, f0:f0 + fw],
                            start=(cc == 0), stop=(cc == 1))
                    t1 = ptmp.tile([104, 512], bf16, name="t1", tag="ptmp")
                    t2 = ptmp.tile([104, 512], bf16, name="t2", tag="ptmp")
                    nc.vector.scalar_tensor_tensor(
                        t1[:, 0:fw], pk[:, 0:fw], qkb_sb[0:104, ch, 2:3],
                        cs_sb[0:104, 0, f0:f0 + fw], op0=OP.add, op1=OP.mult)
                    nc.vector.scalar_tensor_tensor(
                        t2[:, 0:fw], pks[:, 0:fw], qkb_sb[0:104, ch, 3:4],
                        cs_sb[0:104, 1, f0:f0 + fw], op0=OP.add, op1=OP.mult)
                    nc.gpsimd.tensor_add(kt[0:104, b, f0:f0 + fw],
                                         t1[:, 0:fw], t2[:, 0:fw])
                # bias + indicator rows overwrite the zero-padded rows
                kc = KCOLS[ch]
                sync.dma_start(kt[BROW[ch]:BROW[ch] + nh, b, 0:kc],
                               kTbias[b, ch, 0:nh, 0:kc])
                sync.dma_start(kt[BROW[ch] + nh:BROW[ch] + nh + 2, b, 0:kc],
                               kTbias[b, ch, 6:8, 0:kc])

        def vproj(slots):
            for b in range(B):
                for s in slots:
                    hlo = HLO[s]
                    nh_s = 16 - hlo
                    pv = pro.tile([128, 32, 16], f32, name="pv", tag="pro")
                    for cc in range(2):
                        nc.tensor.matmul(pv[:, 0:nh_s, :],
                                         kv_sb[:, b, cc, ts(s, 128)],
                                         wv_sb[:, cc, 16 * hlo:256],
                                         start=(cc == 0), stop=(cc == 1))
                    nc.vector.tensor_copy(
                        vt_sb[:, b, VOFF[s]:VOFF[s] + nh_s, 0:16],
                        pv[:, 0:nh_s, :])

        def attention(pi, b):
            h0 = 2 * pi
            ch = CHUNK_OF_HEAD[h0]
            n = N_P[pi]
            k4 = pi % 4
            kt = KTT[ch]
            K = KCH[ch]
            rhs_t = [None] * n

            def emit_av(si):
                for j in range(2):
                    nc.tensor.matmul(
                        po_t[32 * k4:32 * k4 + 17, j, 0:SQL],
                        vt_sb[:, b, VOFF[si] + (h0 + j - HLO[si]), 0:17],
                        rhs_t[si][:, j, 0:SQL],
                        start=(si == 0), stop=(si == n - 1),
                        skip_group_check=True,
                        tile_position=(0, 32 * k4))

            for si in range(n):
                sc = pscore.tile([128, 2, 512], f32, name="sc", tag="sc")
                for j in range(2):
                    nc.tensor.matmul(
                        sc[:, j, 0:SQL], kt[0:K, b, ts(si, 128)],
                        qTv_sb[0:K, h0 + j, b, 0:SQL],
                        start=True, stop=True)
                ex = pexp.tile([128, 2, SQP], bf16, name="ex", tag="ex")
                nc.scalar.activation(ex[:, :, 0:SQL], sc[:, :, 0:SQL], AF.Exp)
                if KDBG and pi == 2 and b == 1 and si == 1:
                    nc.vector.tensor_copy(dsc_t[:, 0, :, 0:SQL],
                                          sc[:, :, 0:SQL])
                    nc.vector.tensor_copy(dsc_t[:, 1, :, 0:SQL],
                                          ex[:, :, 0:SQL])
                if si in (1, 2):
                    at = pattn.tile([128, 2, SQP], bf16, name="at", tag="at")
                    nc.vector.tensor_mul(at[:, :, 0:SQL], ex[:, :, 0:SQL],
                                         eal_sb[:, pi, si - 1, :, 0:SQL])
                    if KDBG and pi == 2 and b == 1 and si == 1:
                        nc.vector.tensor_copy(dsc_t[:, 2, :, 0:SQL],
                                              at[:, :, 0:SQL])
                    rhs_t[si] = at
                else:
                    rhs_t[si] = ex
                if si >= 1:
                    emit_av(si - 1)
            emit_av(n - 1)

        def close_group(b, g):
            for j in range(2):
                nc.vector.tensor_copy(onum_sb[:, b, g, j, 0:SQL],
                                      po_t[:, j, 0:SQL])

        # ---- emission: prologue chunk-by-chunk, attention b0 then b1 ----
        rope_q()
        rope_k(0, 512)
        qproj(0)
        kproj(0)
        vproj(range(0, 4))
        attention(0, 0)
        attention(1, 0)
        for f0, fw in FCS_CH[2][1:]:
            rope_k(f0, fw)
        qproj(1)
        kproj(1)
        vproj(range(4, 16))
        attention(2, 0)
        attention(3, 0)
        close_group(0, 0)
        qproj(2)
        kproj(2)
        vproj([16])
        # halo mask broadcast [1,SQL] -> [128,SQL]
        pmh = pro.tile([128, SQL], f32, name="pmh", tag="pro")
        nc.tensor.matmul(pmh[:, :], onesBb, hm_sb[:, :], start=True, stop=True)
        nc.scalar.activation(maskB_sb[:, :], pmh[:, :], AF.Copy)
        attention(4, 0)
        attention(5, 0)
        attention(6, 0)
        attention(7, 0)
        close_group(0, 1)
        es_pro.close()
        rm_es.close()

        pdiv = ExitStack()
        pd = pdiv.enter_context(
            tc.tile_pool(name="pdiv", bufs=2, space="PSUM", side="right"))
        pt_es = ExitStack()
        pt = pt_es.enter_context(tc.tile_pool(name="tail_sb", bufs=2))

        def div(b):
            for g in range(2):
                for j in range(2):
                    pden = pd.tile([4, 512], f32, name="pden", tag="a")
                    nc.tensor.matmul(pden[:, 0:SQL], sel4_sb[:, :],
                                     onum_sb[:, b, g, j, 0:SQL],
                                     start=True, stop=True)
                    nc.vector.tensor_copy(denf4[0:4, b, g, j, 0:SQL],
                                          pden[0:4, 0:SQL])
            for g in range(2):
                nc.vector.reciprocal_approx_fast(recf4[0:4, b, g, :, 0:SQL],
                                                 denf4[0:4, b, g, :, 0:SQL])
                nc.vector.tensor_copy(rec4[0:4, b, g, :, 0:SQL],
                                      recf4[0:4, b, g, :, 0:SQL])
            for g in range(2):
                for j in range(2):
                    recB = pd.tile([128, 512], f32, name="recB", tag="a")
                    nc.tensor.matmul(
                        recB[:, 0:SQL], ind4_sb[0:4, :],
                        rec4[0:4, b, g, j, 0:SQL],
                        start=True, stop=True)
                    nc.vector.tensor_mul(oall_sb[:, b, g, j, 0:SQL],
                                         onum_sb[:, b, g, j, 0:SQL],
                                         recB[:, 0:SQL])

        def oproj(b):
            x1p = []
            for pc in range(2):
                px = pd.tile([128, 512], f32, name="px", tag="a")
                for g in range(2):
                    for j in range(2):
                        nc.tensor.matmul(
                            px[:, 0:SQL], wop_sb[:, g, j, ts(pc, 128)],
                            oall_sb[:, b, g, j, 0:SQL],
                            start=(g == 0 and j == 0),
                            stop=(g == 1 and j == 1))
                xt = pt.tile([128, SQL], bf16, name="x1p", tag="x1p")
                nc.vector.scalar_tensor_tensor(
                    xt[:, :], px[:, 0:SQL], tv(pc, 0), qsrc_sb[:, b, pc, :],
                    op0=OP.add, op1=OP.add)
                x1p.append(xt)
            return x1p

        def newton_rsqrt(vee, F):
            """rstd [1,F] f32r from vee [1,F] f32 via bit-trick + 2 Newton."""
            ti = pt.tile([1, SQL], i32, name="nr_ti", tag="nr_i")
            nc.vector.tensor_scalar(ti[:, 0:F], vee.bitcast(i32), 1, None,
                                    op0=OP.logical_shift_right)
            tn = pt.tile([1, SQL], i32, name="nr_tn", tag="nr_f")
            nc.vector.tensor_scalar(tn[:, 0:F], ti[:, 0:F], -1, None,
                                    op0=OP.bitwise_xor)
            y0b = pt.tile([1, SQL], i32, name="nr_y0", tag="nr_i")
            nc.vector.tensor_scalar(y0b[:, 0:F], tn[:, 0:F], 0x5f3759e0, None,
                                    op0=OP.add)
            y = y0b[:, 0:F].bitcast(f32)
            for it in range(2):
                t = pt.tile([1, SQL], f32, name="nr_t", tag="nr_f")
                nc.vector.tensor_mul(t[:, 0:F], y, y)
                h = pt.tile([1, SQL], f32, name="nr_h", tag="nr_f")
                nc.vector.scalar_tensor_tensor(h[:, 0:F], vee, -0.5, t[:, 0:F],
                                               op0=OP.mult, op1=OP.mult)
                yn = pt.tile([1, SQL], f32r if it == 1 else f32,
                             name="nr_y", tag="nr_y")
                nc.vector.scalar_tensor_tensor(yn[:, 0:F], h[:, 0:F], 1.5, y,
                                               op0=OP.add, op1=OP.mult)
                y = yn[:, 0:F]
            return y

        def layernorm(xin, F, gi, bi, pool, odt=bf16, sq_act=False,
                      otag="ln_o", obufs=2):
            """LN over channel dim (256 = partitions across 2 chunks)."""
            sqs = []
            for pc in range(2):
                sq = pt.tile([128, SQL], bf16, name="ln_sq", tag="ln_sq")
                if sq_act:
                    nc.scalar.activation(sq[:, 0:F], xin[pc][:, 0:F], AF.Square)
                else:
                    nc.gpsimd.tensor_mul(sq[:, 0:F], xin[pc][:, 0:F],
                                         xin[pc][:, 0:F])
                sqs.append(sq)
            ps = pool.tile([1, 512], f32, name="ln_ps", tag="a", bufs=2)
            for pc in range(2):
                nc.tensor.matmul(ps[:, 0:F], ones128b, xin[pc][:, 0:F],
                                 start=(pc == 0), stop=(pc == 1))
            ps2 = pool.tile([1, 512], f32, name="ln_ps2", tag="a", bufs=2)
            for pc in range(2):
                nc.tensor.matmul(ps2[:, 0:F], ones128b, sqs[pc][:, 0:F],
                                 start=(pc == 0), stop=(pc == 1))
            mean = pt.tile([1, SQL], f32r, name="ln_mean", tag="ln_mean")
            nc.vector.tensor_scalar(mean[:, 0:F], ps[:, 0:F], 1.0 / D, None,
                                    op0=OP.mult)
            em2 = pt.tile([1, SQL], f32, name="ln_em2", tag="ln_em2")
            nc.vector.scalar_tensor_tensor(em2[:, 0:F], mean[:, 0:F], -1.0,
                                           mean[:, 0:F], op0=OP.mult,
                                           op1=OP.mult)
            var = pt.tile([1, SQL], f32, name="ln_var", tag="ln_var")
            nc.vector.scalar_tensor_tensor(var[:, 0:F], ps2[:, 0:F], 1.0 / D,
                                           em2[:, 0:F], op0=OP.mult, op1=OP.add)
            vee = pt.tile([1, SQL], f32, name="ln_vee", tag="ln_vee")
            nc.vector.tensor_scalar(vee[:, 0:F], var[:, 0:F], 1e-5, None,
                                    op0=OP.add)
            rstd = newton_rsqrt(vee[:, 0:F], F)
            pmb = pool.tile([128, 512], f32, name="ln_pmb", tag="a", bufs=2)
            nc.tensor.matmul(pmb[:, 0:F], onesBr, mean[:, 0:F],
                             start=True, stop=True)
            prb = pool.tile([128, 512], f32, name="ln_prb", tag="a", bufs=2)
            nc.tensor.matmul(prb[:, 0:F], onesBr, rstd, start=True, stop=True)
            outs = []
            for pc in range(2):
                t = pt.tile([128, SQL], bf16, name="ln_t", tag="ln_t")
                nc.vector.tensor_sub(t[:, 0:F], xin[pc][:, 0:F], pmb[:, 0:F])
                t2 = pt.tile([128, SQL], bf16, name="ln_t2", tag="ln_t2")
                nc.vector.tensor_mul(t2[:, 0:F], t[:, 0:F], prb[:, 0:F])
                o = pt.tile([128, SQL], odt, name="ln_o", tag=otag,
                            bufs=obufs)
                nc.scalar.activation(o[:, 0:F], t2[:, 0:F], AF.Identity,
                                     bias=tv(pc, bi), scale=tv(pc, gi))
                outs.append(o)
            return outs

        # b0 attention done; div/oproj/LN1/LN2 of b0 overlap b1 attention.
        div(0)
        x1p0 = oproj(0)
        attention(0, 1)
        attention(1, 1)
        x1_0 = layernorm(x1p0, SQL, 1, 2, pd, otag="x1")
        attention(2, 1)
        attention(3, 1)
        close_group(1, 0)
        c0_0 = layernorm(x1_0, SQL, 3, 4, pd, odt=f32r, otag="c0",
                         obufs=4)
        attention(4, 1)
        attention(5, 1)
        attention(6, 1)
        attention(7, 1)
        close_group(1, 1)
        div(1)
        x1p1 = oproj(1)

        # attention fully done: free score/po PSUM, open the wide tail pool
        es_attn.close()
        pt3_es = ExitStack()
        pt3 = pt3_es.enter_context(
            tc.tile_pool(name="tail_psum", bufs=4, space="PSUM"))

        x1_1 = layernorm(x1p1, SQL, 1, 2, pt3, sq_act=True, otag="x1")
        c0_1 = layernorm(x1_1, SQL, 3, 4, pt3, odt=f32r, sq_act=True,
                         otag="c0", obufs=4)
        c0 = {0: c0_0, 1: c0_1}

        def mm4(widx, rhs_tiles, F, pool):
            outs = []
            for pc in range(2):
                p = pool.tile([128, 512], f32, name="mm4", tag="t")
                for cc in range(2):
                    nc.tensor.matmul(p[:, 0:F],
                                     tw_sb[:, widx, cc, ts(pc, 128)],
                                     rhs_tiles[cc][:, 0:F],
                                     start=(cc == 0), stop=(cc == 1))
                outs.append(p)
            return outs

        # pw1 + gelu (first gelu-set op; single table switch lives here)
        cm = {}
        for b in range(B):
            cp = mm4(1, c0[b], SQL, pt3)
            cmb = []
            for pc in range(2):
                cg = pt.tile([128, SQL], bf16, name="cg", tag="cg")
                nc.scalar.activation(cg[:, :], cp[pc][:, 0:SQL], AF.Gelu,
                                     bias=tv(pc, 5))
                cmt = pt.tile([128, SQL], bf16, name="cmt", tag="cmt", bufs=4)
                nc.gpsimd.tensor_mul(cmt[:, :], cg[:, :], maskB_sb[:, :])
                cmb.append(cmt)
            cm[b] = cmb
        # depthwise conv (3 taps) + BN + hardswish
        hsw = {}
        for b in range(B):
            hswb = []
            for pc in range(2):
                cmp_ = cm[b][pc]
                a1 = pt.tile([128, WIN], bf16, name="a1", tag="a1")
                nc.vector.tensor_scalar(a1[:, :], cmp_[:, 1:WIN + 1],
                                        tv(pc, 7), None, op0=OP.mult)
                a2 = pt.tile([128, WIN], bf16, name="a2", tag="a2")
                nc.vector.scalar_tensor_tensor(
                    a2[:, :], cmp_[:, 0:WIN], tv(pc, 6), a1[:, :],
                    op0=OP.mult, op1=OP.add)
                a3 = pt.tile([128, WIN], bf16, name="a3", tag="a3")
                nc.vector.scalar_tensor_tensor(
                    a3[:, :], cmp_[:, 2:WIN + 2], tv(pc, 8), a2[:, :],
                    op0=OP.mult, op1=OP.add)
                bn = pt.tile([128, WIN], bf16, name="bn", tag="bn")
                nc.scalar.activation(bn[:, :], a3[:, :], AF.Identity,
                                     bias=tv(pc, 10), scale=tv(pc, 9))
                h1 = pt.tile([128, WIN], bf16, name="h1", tag="h1")
                nc.vector.tensor_scalar(h1[:, :], bn[:, :], 3.0, 6.0,
                                        op0=OP.add, op1=OP.min)
                h2 = pt.tile([128, WIN], bf16, name="h2", tag="h2")
                nc.scalar.activation(h2[:, :], h1[:, :], AF.Relu,
                                     scale=1.0 / 6.0)
                hst = pt.tile([128, WIN], f32r, name="hst", tag="hst", bufs=4)
                nc.vector.tensor_mul(hst[:, :], bn[:, :], h2[:, :])
                hswb.append(hst)
            hsw[b] = hswb
        # pw2 (+bias) then FFN
        x2 = {}
        for b in range(B):
            p2 = mm4(2, hsw[b], WIN, pt3)
            x2b = []
            for pc in range(2):
                x2t = pt.tile([128, WIN], f32r, name="x2t", tag="x2t", bufs=4)
                nc.scalar.activation(x2t[:, :], p2[pc][:, 0:WIN], AF.Identity,
                                     bias=tv(pc, 11))
                x2b.append(x2t)
            x2[b] = x2b
        gg = {}
        for b in range(B):
            p3 = mm4(3, x2[b], WIN, pt3)
            ggb = []
            for pc in range(2):
                g1 = pt.tile([128, WIN], f32r, name="g1", tag="g1", bufs=4)
                nc.scalar.activation(g1[:, :], p3[pc][:, 0:WIN], AF.Gelu,
                                     bias=tv(pc, 12))
                ggb.append(g1)
            gg[b] = ggb
        for b in range(B):
            p4 = mm4(4, gg[b], WIN, pt3)
            x3 = []
            for pc in range(2):
                x3t = pt.tile([128, WIN], bf16, name="x3t", tag="x3t")
                nc.vector.scalar_tensor_tensor(
                    x3t[:, :], p4[pc][:, 0:WIN], tv(pc, 13), x2[b][pc][:, :],
                    op0=OP.add, op1=OP.add)
                x3.append(x3t)
            xo = layernorm(x3, WIN, 14, 15, pt3, odt=f32, sq_act=True,
                           otag="xo")
            for pc in range(2):
                sync.dma_start(out[b, ts(pc, 128), :], xo[pc][:, 0:WIN])

        if KDBG:
            sync.dma_start(d_onum[:, :, :, :, :], onum_sb[:, :, :, :, :])
            sync.dma_start(d_oall[:, :, :, :, :], oall_sb[:, :, :, :, :])
            sync.dma_start(d_den[:, :, :, :], den36[:, :, :, :])
            sync.dma_start(d_rec[:, :, :, :], recf36[:, :, :, :])
            sync.dma_start(d_qtv[:, :, :, :], qTv_sb[:, :, :, :])
            sync.dma_start(d_kt0[:, :, :], kT0[:, :, :])
            sync.dma_start(d_vt[:, :, :, :], vt_sb[:, :, :, :])
            sync.dma_start(d_eal[:, :, :, :, :], eal_sb[:, :, :, :, :])
            sync.dma_start(d_sc[:, :, :, :], dsc_t[:, :, :, :])
        pdiv.close()
        pt3_es.close()
        pt_es.close()
        for _t, _free in reversed(_keep):
            _free()

    nc.compile()
    return nc


# ---------------------------------------------------------------------------
# Host-side input prep (sharding)
# ---------------------------------------------------------------------------

def host_prep(inputs):
    f32 = np.float32

    src = np.asarray(inputs["src"], f32)
    alibi = np.asarray(inputs["alibi_bias"], f32)
    pos_emb = np.asarray(inputs["pos_emb"], f32)
    mask = np.asarray(inputs["mask"])

    slopes = -alibi[:, 0, 1].astype(np.float64)  # alibi[h,0,1] = -slope_h
    exp_slopes = 2.0 ** (-8.0 * (np.arange(H) + 1) / H)
    assert np.allclose(slopes, exp_slopes, rtol=1e-3), "unexpected alibi slopes"

    cos = np.cos(pos_emb).astype(f32)  # [S, 8]
    sin = np.sin(pos_emb).astype(f32)

    # swap perm within each head: j -> (j+8)%16
    jj = np.arange(D)
    swap = (jj // HD) * HD + (jj % HD + HD // 2) % HD

    wq, wk, wvm = [np.asarray(inputs[k], f32) for k in ("wq", "wk", "wv")]
    bq, bk, bv = [np.asarray(inputs[k], f32) for k in ("bq", "bk", "bv")]

    # chunked weight cols, zero-padded to 104 (defines kT/qT rows 96:104)
    def chunk_cols(m):  # m [D, D] -> [D, 3, 104]
        outm = np.zeros((D, 3, 104), f32)
        for ch in range(3):
            w = 16 * NH[ch]
            outm[:, ch, 0:w] = m[:, 96 * ch:96 * ch + w]
        return outm

    wq2 = np.stack([chunk_cols(wq.T), chunk_cols(wq.T[:, swap])]).astype(BF16)
    wk2 = np.stack([chunk_cols(wk.T), chunk_cols(wk.T[:, swap])]).astype(BF16)

    # qkb [128, 3, 4]: chunk-local rows
    qkb = np.zeros((128, 3, 4), f32)
    for ch in range(3):
        w = 16 * NH[ch]
        sl = slice(96 * ch, 96 * ch + w)
        qkb[0:w, ch, 0] = bq[sl]
        qkb[0:w, ch, 1] = bq[swap][sl]
        qkb[0:w, ch, 2] = bk[sl]
        qkb[0:w, ch, 3] = bk[swap][sl]

    # Mcos[i, v, r]: cos-select (r%8==i); Msin adds sign by half
    r = np.arange(128)
    mc = (r[None, :] % 8 == np.arange(8)[:, None]).astype(f32)
    sgn_r = np.where((r % HD) < HD // 2, -1.0, 1.0).astype(f32)
    ropem = np.ascontiguousarray(
        np.stack([mc, mc * sgn_r[None, :]], 1)).astype(BF16)

    # head-slot row masks [128, 16]
    pm = np.zeros((128, 16), f32)
    for h in range(16):
        ch = CHUNK_OF_HEAD[h]
        i = h - HBASE[ch]
        pm[16 * i:16 * i + 16, h] = 1.0

    # qTv bias rows [16, 8, SQL]
    fidx = np.arange(SQL, dtype=f32)
    qvr = np.zeros((16, 8, SQL), f32)
    for h in range(16):
        ch = CHUNK_OF_HEAD[h]
        nh = NH[ch]
        i = h - HBASE[ch]
        sl_f = np.float32(slopes[h])
        qvr[h, i, :] = 1.0
        qvr[h, nh, :] = (-sl_f * fidx).astype(BF16).astype(f32)
        qvr[h, nh + 1, :] = (-sl_f * (257.0 - fidx)).astype(BF16).astype(f32)
    qvr = qvr.astype(BF16)

    # tail weights / vectors
    wo, pw1, pw2, w1m, w2m = [np.asarray(inputs[k], f32)
                              for k in ("wo", "pw1_w", "pw2_w", "w1", "w2")]
    tailw = np.ascontiguousarray(
        np.stack([wo.T, pw1.T, pw2.T, w1m.T, w2m.T])).astype(f32)
    dww = np.asarray(inputs["dw_w"], f32)  # [D, 1, 3]
    sbn = (np.asarray(inputs["bn_g"], f32) /
           np.sqrt(np.asarray(inputs["bn_var"], f32) + 1e-5))
    tbn = ((np.asarray(inputs["dw_b"], f32) -
            np.asarray(inputs["bn_mean"], f32)) * sbn +
           np.asarray(inputs["bn_b"], f32))
    bo2 = np.asarray(inputs["bo"], f32) + wo @ bv   # vproj bias fold
    vecs = [bo2, inputs["n1_g"], inputs["n1_b"], inputs["ln_g"],
            inputs["ln_b"], inputs["pw1_b"], dww[:, 0, 0], dww[:, 0, 1],
            dww[:, 0, 2], sbn, tbn, inputs["pw2_b"], inputs["b1"],
            inputs["b2"], inputs["n2_g"], inputs["n2_b"]]
    tailv = np.stack([np.asarray(v, f32) for v in vecs], -1)  # [D, 16]
    tailv = np.ascontiguousarray(
        tailv.reshape(2, 128, 16).transpose(1, 0, 2)).astype(f32)

    # wop [2, 2, 128, D]: oall row p of (g, j) -> head 2*(4g+p//32)+j, hd p%32
    wopt = np.zeros((2, 2, 128, D), f32)
    p128 = np.arange(128)
    for g in range(2):
        for j in range(2):
            h_of = 2 * (4 * g + p128 // 32) + j
            hd = p128 % 32
            real = hd < 16
            wopt[g, j, real, :] = wo.T[(h_of * 16 + hd)[real], :]

    # ind4 [4, 128]: recB[p] = rec[p//32];  sel4 [128, 4]: den row selector
    ind4 = np.zeros((4, 128), f32)
    for rr in range(4):
        ind4[rr, p128 // 32 == rr] = 1.0
    sel4 = np.zeros((128, 4), f32)
    for k in range(4):
        sel4[32 * k + 16, k] = 1.0

    maskvec = mask.astype(bool)  # [B, S]

    in_maps = []
    for c in range(NCORES):
        q0 = c * WIN - 1
        wb = 2 * c - 1
        diag = [wb, wb + 1, wb + 2, wb + 3]

        def mindist(g):
            return max(q0 - (128 * g + 127), 128 * g - (q0 + 257), 0)

        rest = sorted((g for g in range(NB) if g not in diag), key=mindist)
        perm = [(g if 0 <= g < NB else -1) for g in diag] + rest
        perm = perm + [-1] * (NSLOT - len(perm))

        # permuted / padded per-core tensors
        srckv_c = np.zeros((B, D, SK), f32)
        ropecs_c = np.zeros((8, 2, SK), f32)
        for s, g in enumerate(perm):
            if g < 0:
                continue
            sl = slice(128 * s, 128 * s + 128)
            gsl = slice(128 * g, 128 * g + 128)
            srckv_c[:, :, sl] = src.transpose(0, 2, 1)[:, :, gsl]
            ropecs_c[:, 0, sl] = cos[gsl, :].T
            ropecs_c[:, 1, sl] = sin[gsl, :].T

        # kT bias rows [B, 3, 8, SK]
        kTb = np.zeros((B, 3, 8, SK), f32)
        skpos = np.zeros(SK, np.int64)
        isdum = np.zeros(SK, bool)
        for s, g in enumerate(perm):
            sl = slice(128 * s, 128 * s + 128)
            if g < 0:
                isdum[sl] = True
            else:
                skpos[sl] = 128 * g + p128
        edged = np.maximum(np.maximum(q0 - skpos, skpos - (q0 + 257)), 0
                           ).astype(f32)
        indL = ((skpos <= q0) & ~isdum).astype(f32)
        indR = ((skpos >= q0 + 257) & ~isdum).astype(f32)
        for b in range(B):
            mrow = np.where(maskvec[b][np.clip(skpos, 0, S - 1)], -30.0, 0.0
                            ).astype(f32)
            for ch in range(3):
                for i in range(NH[ch]):
                    h = HBASE[ch] + i
                    bias_row = (-np.float32(slopes[h]) * edged) + mrow
                    bias_row[isdum] = -30.0
                    kTb[b, ch, i, :] = bias_row
                kTb[b, ch, 6, :] = indL
                kTb[b, ch, 7, :] = indR
        kTb_bf = kTb.astype(BF16)

        # diag exp(alibi) for the two fully-in-window slots (perm[1], perm[2])
        eald_c = np.ones((8, 128, 2, 2, SQP), f32)
        qpos = np.clip(q0 + np.arange(SQL), 0, S - 1)
        for pi in range(8):
            for si2 in range(2):
                g = perm[1 + si2]
                sk = 128 * g + p128
                for j in range(2):
                    h = 2 * pi + j
                    eald_c[pi, :, si2, j, 0:SQL] = np.exp(
                        np.minimum(alibi[h][np.ix_(sk, qpos)], 0.0))

        # q-window slice with zero padding
        sq = np.zeros((B, SQL, D), f32)
        lo, hi = max(q0, 0), min(q0 + SQL, S)
        sq[:, lo - q0:hi - q0, :] = src[:, lo:hi, :]
        srcqT = sq.transpose(0, 2, 1)  # [B, D, SQL]

        csq = np.zeros((8, 2, SQL), f32)
        csq[:, 0, lo - q0:hi - q0] = 0.25 * cos[lo:hi, :].T
        csq[:, 1, lo - q0:hi - q0] = 0.25 * sin[lo:hi, :].T

        halo = np.ones((1, SQL), f32)
        if q0 < 0:
            halo[0, 0] = 0.0
        if q0 + SQL > S:
            halo[0, SQL - 1] = 0.0

        in_maps.append({
            "srckv": srckv_c.astype(BF16),
            "srcq": srcqT.astype(BF16),
            "wq2": wq2, "wk2": wk2,
            "wv": wvm.T.astype(BF16),
            "ropecs": ropecs_c.astype(BF16),
            "ropecsq": csq.astype(BF16),
            "ropem": ropem,
            "qkb": qkb,
            "kTbias": kTb_bf,
            "qvr": qvr,
            "eald": eald_c.astype(BF16),
            "tailw": tailw,
            "tailv": tailv,
            "halom": halo.astype(BF16),
            "pmask": pm,
            "onesbf": np.ones((128, 128), BF16),
            "onesr": np.ones((128, 128), f32),
            "wop": wopt.astype(BF16),
            "ind4": ind4,
            "sel4": sel4.astype(BF16),
        })
    return in_maps


def kernel(**inputs) -> np.ndarray:
    _ensure_ntff_hook()
    from concourse.bass_utils import run_bass_kernel_spmd

    if "nc" not in _COMPILED:
        _COMPILED["nc"] = build_nc()
    nc = _COMPILED["nc"]

    in_maps = host_prep(inputs)
    trace = os.environ.get("KERNEL_TRACE", "0") == "1"
    res = run_bass_kernel_spmd(nc, in_maps, core_ids=list(range(NCORES)),
                               trace=trace)
    kernel.last_result = res

    b_, s_, d_ = inputs["src"].shape
    full = np.empty((b_, s_, d_), np.float32)
    for c in range(NCORES):
        o = res.results[c]["out"]  # [B, D, win]
        full[:, c * WIN:(c + 1) * WIN, :] = o.transpose(0, 2, 1)
    return full


# revision 29
# speedup vs baseline: 1.2786x; 1.0555x over previous
"""Trainium2 Bass kernel for ALiBiConformerEncoderLayer (8-core SPMD).

Sharding: sequence windows (256 queries + 1 halo col each side) per core.

v2 redesign vs the 311us baseline:
- No big memsets: matmul contraction restricted to K=104/70 rows (weights
  zero-padded to 104 cols), vt packed per-slot with static ones column.
- vproj bias matmul dropped (bo' = bo + wo@bv host fold); per-slot head
  restriction (only heads whose pair reaches that slot).
- kproj column restriction per chunk (512 / 2048 / 2176).
- Inclusive indicator rows (sk<=q0, sk>=q0+257) make the factorized alibi
  exact on the window-boundary rows: only 2 diag slots need the 2D
  exp(alibi) multiply (eald halved).
- AV matmuls land 4 pairs per PSUM tile via tile_position col offsets;
  division epilogue = 8 PSUM->SBUF copies + tiny den DMAs + 4-row
  indicator matmul broadcast + 8 muls (was 16 copies + 32 bcasts + 32 muls).
- b-split attention: all pairs b0, then b1; b0's oproj/LN1/LN2 overlap b1's
  attention. Act-table discipline: only {exp, square, identity, copy, relu}
  until the single gelu-set switch (greedy table loader thrashes otherwise).
- LN rsqrt via DVE Newton iteration (bit-trick seed) - no Ln/Sqrt tables.
"""
import os
import sys
import types
from contextlib import ExitStack

import numpy as np
import ml_dtypes

BF16 = ml_dtypes.bfloat16

# Problem constants (hardcoded; kernel.py must be self-contained)
B, S, D, H, HD = 2, 2048, 256, 16, 16
NCORES = 8
WIN = S // NCORES          # 256
SQL = WIN + 2              # query cols incl 1 halo each side
SQP = 260                  # 4-elem aligned bf16 stride
NB = S // 128              # 16 real sk blocks
NSLOT = 17                 # 16 real + 1 dummy (window-centered permutation)
SK = NSLOT * 128

# kT chunking: heads per chunk, base head of chunk
NH = [6, 6, 4]
HBASE = [0, 6, 12]
CHUNK_OF_HEAD = [0] * 6 + [1] * 6 + [2] * 4
KCH = [104, 104, 70]       # contraction rows per chunk (vals+bias+ind)
BROW = [96, 96, 64]        # first bias row per chunk (16*nh)
QVR_N = [8, 8, 6]          # qTv bias rows per head
KCOLS = [512, 2048, 2176]  # kT cols needed per chunk

# Block-skip: pair pi covers heads (2pi, 2pi+1); reach D = T/slope(2pi+1).
N_P = [4, 4, 4, 4, 6, 10, 17, 17]

# vt packing: first head needing slot s, offsets
HLO = [0] * 4 + [8] * 2 + [10] * 4 + [12] * 7
VOFF = [0] * NSLOT
for _s in range(1, NSLOT):
    VOFF[_s] = VOFF[_s - 1] + (16 - HLO[_s - 1])
NVT = VOFF[-1] + (16 - HLO[-1])  # 156

FCS_CH = [
    [(0, 512)],
    [(0, 512), (512, 512), (1024, 512), (1536, 512)],
    [(0, 512), (512, 512), (1024, 512), (1536, 512), (2048, 128)],
]

_COMPILED = {}


def _ensure_ntff_hook():
    """Install the axon NTFF profiling hook if the image lacks antenv.axon_hooks."""
    try:
        import antenv.axon_hooks  # noqa: F401
        return
    except ImportError:
        pass
    try:
        from trn_agent_boot.trn_boot import _ntff_profile_via_ctypes
        hook = _ntff_profile_via_ctypes('/opt/axon/libaxon_pjrt.so')
    except Exception:
        hook = None
    mod = types.ModuleType('antenv.axon_hooks')
    mod.get_axon_ntff_profile_hook = lambda: hook
    mod.set_axon_ntff_profile_hook = lambda h: None
    sys.modules['antenv.axon_hooks'] = mod


# ---------------------------------------------------------------------------
# Graph builder
# ---------------------------------------------------------------------------

def build_nc():
    import concourse.bass as bass  # noqa: F401
    import concourse.tile as tile
    from concourse import bacc, mybir
    from concourse.bass import ts

    f32 = mybir.dt.float32
    f32r = mybir.dt.float32r
    bf16 = mybir.dt.bfloat16
    i32 = mybir.dt.int32
    AF = mybir.ActivationFunctionType
    OP = mybir.AluOpType

    nc = bacc.Bacc(None, target_bir_lowering=False)

    P = {}
    def dram(name, shape, dt):
        P[name] = nc.declare_dram_parameter(name, list(shape), dt, isOutput=False)
        return P[name]

    srckv = dram("srckv", [128, B, 2, SK], bf16)    # permuted src.T
    srcq = dram("srcq", [128, B, 2, SQL], bf16)
    wqk = dram("wqk", [128, 2, 2, 2, 3, 104], bf16)  # (qk, v, cc) chunked
    wv = dram("wv", [128, 2, D], bf16)              # wvT (bias folded into bo')
    ropecs = dram("ropecs", [8, 2, SK], bf16)       # cosT, sinT (k side, permuted)
    ropecsq = dram("ropecsq", [8, 2, SQL], bf16)    # 0.25*cos/sin (q side)
    ropem = dram("ropem", [8, 2, 128], bf16)        # Mcos, Msin(signed)
    qkb = dram("qkb", [128, 3, 4], f32)             # bq, bqs, bk, bks chunk rows
    kTbias = dram("kTbias", [B, 3, 8, SK], bf16)    # B rows (0:nh) + ind (6:8)
    qvr = dram("qvr", [8, 16, 2, SQL], bf16)        # qTv bias rows per head
    eald = dram("eald", [128, 8, 2, 2, SQP], bf16)  # diag exp(alibi), slots 1,2
    tailw = dram("tailw", [128, 5, 2, D], f32r)     # woT pw1T pw2T w1T w2T
    tailv = dram("tailv", [128, 2, 16], f32)
    halom = dram("halom", [1, SQL], bf16)
    pmask = dram("pmask", [128, 16], f32)           # head-slot row masks
    onesbf = dram("onesbf", [128, 128], bf16)
    onesr = dram("onesr", [128, 128], f32r)
    wop = dram("wop", [128, 2, 2, D], bf16)         # woT rows per (grp, j)
    ind4 = dram("ind4", [4, 128], f32r)             # recB indicator rows
    sel4 = dram("sel4", [128, 4], bf16)             # den-row selector
    out = nc.declare_dram_parameter("out", [B, D, WIN], f32, isOutput=True)
    KDBG = os.environ.get("KDBG", "0") == "1"
    if KDBG:
        d_onum = nc.declare_dram_parameter("d_onum", [128, B, 2, 2, SQP], bf16,
                                           isOutput=True)
        d_oall = nc.declare_dram_parameter("d_oall", [128, B, 2, 2, SQP], bf16,
                                           isOutput=True)
        d_den = nc.declare_dram_parameter("d_den", [36, B, 2, SQP], bf16,
                                          isOutput=True)
        d_rec = nc.declare_dram_parameter("d_rec", [36, B, 2, SQP], f32,
                                          isOutput=True)
        d_qtv = nc.declare_dram_parameter("d_qtv", [128, 16, B, SQP], bf16,
                                          isOutput=True)
        d_kt0 = nc.declare_dram_parameter("d_kt0", [128, B, 512], bf16,
                                          isOutput=True)
        d_vt = nc.declare_dram_parameter("d_vt", [128, B, NVT, 20], bf16,
                                         isOutput=True)
        d_eal = nc.declare_dram_parameter("d_eal", [128, 8, 2, 2, SQP], bf16,
                                          isOutput=True)
        d_sc = nc.declare_dram_parameter("d_sc", [128, 3, 2, SQP], f32,
                                         isOutput=True)

    with ExitStack() as top, tile.TileContext(nc) as tc:
        _keep = []
        def ctile(shape, dt, name):
            t, _free = tc.tile(list(shape), dt, name=name)
            _keep.append((t, _free))
            return t

        sync = nc.sync

        # ---- persistent SBUF ----
        tw_sb = ctile([128, 5, 2, D], f32r, "tw_sb")
        tv_sb = ctile([128, 2, 16], f32, "tv_sb")
        qkb_sb = ctile([128, 3, 4], f32, "qkb_sb")
        pm_sb = ctile([128, 16], f32, "pm_sb")
        onesbf_sb = ctile([128, 128], bf16, "onesbf_sb")
        onesr_sb = ctile([128, 128], f32r, "onesr_sb")
        ind4_sb = ctile([4, 128], f32r, "ind4_sb")
        sel4_sb = ctile([128, 4], bf16, "sel4_sb")
        wop_sb = ctile([128, 2, 2, D], bf16, "wop_sb")
        qsrc_sb = ctile([128, B, 2, SQL], bf16, "qsrc_sb")
        kT0 = ctile([128, B, 512], bf16, "kT0")
        kT1 = ctile([128, B, 2048], bf16, "kT1")
        kT2 = ctile([128, B, SK], bf16, "kT2")
        KTT = [kT0, kT1, kT2]
        qTv_sb = ctile([128, 16, B, SQP], bf16, "qTv_sb")
        vt_sb = ctile([128, B, NVT, 20], bf16, "vt_sb")
        eal_sb = ctile([128, 8, 2, 2, SQP], bf16, "eal_sb")
        onum_sb = ctile([128, B, 2, 2, SQP], bf16, "onum_sb")
        oall_sb = ctile([128, B, 2, 2, SQP], bf16, "oall_sb")
        denf4 = ctile([4, B, 2, 2, SQP], f32, "denf4")
        recf4 = ctile([4, B, 2, 2, SQP], f32, "recf4")
        rec4 = ctile([4, B, 2, 2, SQP], f32r, "rec4")
        maskB_sb = ctile([128, SQL], bf16, "maskB_sb")
        eps1 = ctile([1, 2], f32, "eps1")
        dsc_t = ctile([128, 3, 2, SQP], f32, "dsc_t") if KDBG else None

        nc.vector.memset(eps1[:, :], 1e-5)
        # Act warmup: force the exp table load at t=0 (first Act op)
        nc.scalar.activation(eps1[:, 1:2], eps1[:, 0:1], AF.Exp)

        # ---- initial DMA: priority order ----
        # prologue-only SBUF (weights, rope tables, kv): freed before the
        # tail pool opens so left/right SBUF stacks never collide.
        rm_es = ExitStack()
        p1c = rm_es.enter_context(tc.tile_pool(name="p1c", bufs=1))
        def p1tile(shape, dt, name):
            return p1c.tile(list(shape), dt, name=name, tag=name)

        wqk_sb = p1tile([128, 2, 2, 2, 3, 104], bf16, "wqk_sb")
        wq_sb = wqk_sb[:, 0]
        wk_sb = wqk_sb[:, 1]
        wv_sb = p1tile([128, 2, D], bf16, "wv_sb")
        rm_sb = p1tile([8, 2, 128], bf16, "rm_sb")
        rcs_sb = p1tile([8, 2, SK], bf16, "rcs_sb")
        rcsq_sb = p1tile([8, 2, SQL], bf16, "rcsq_sb")
        kv_sb = p1tile([128, B, 2, SK], bf16, "kv_sb")
        cs_sb = p1tile([128, 2, SK], bf16, "cs_sb")
        csq_sb = p1tile([128, 2, SQL], bf16, "csq_sb")
        qT_sb = p1tile([128, B, 3, SQL], bf16, "qT_sb")
        hm_sb = p1tile([1, SQL], bf16, "hm_sb")

        sync.dma_start(rm_sb[:, :, :], ropem[:, :, :])
        sync.dma_start(rcsq_sb[:, :, :], ropecsq[:, :, :])
        sync.dma_start(rcs_sb[:, :, :], ropecs[:, :, :])
        sync.dma_start(qkb_sb[:, :, :], qkb[:, :, :])
        sync.dma_start(pm_sb[:, :], pmask[:, :])
        sync.dma_start(onesbf_sb[:, :], onesbf[:, :])
        sync.dma_start(onesr_sb[:, :], onesr[:, :])
        sync.dma_start(qsrc_sb[:, :, :, :], srcq[:, :, :, :])
        sync.dma_start(wqk_sb[:, :, :, :, :, :], wqk[:, :, :, :, :, :])
        # srckv: diag slots (cols 0:512) first so vproj/attn(ch0) start early
        sync.dma_start(kv_sb[:, :, :, 0:512], srckv[:, :, :, 0:512])
        sync.dma_start(wv_sb[:, :, :], wv[:, :, :])
        sync.dma_start(eal_sb[:, 0:3, :, :, :], eald[:, 0:3, :, :, :])
        # qTv bias rows: disjoint from all engine writes, safe at startup
        sync.dma_start(qTv_sb[96:104, 0:12, :, 0:SQL], qvr[0:8, 0:12, :, :])
        sync.dma_start(qTv_sb[64:70, 12:16, :, 0:SQL], qvr[0:6, 12:16, :, :])
        sync.dma_start(kv_sb[:, :, :, 512:SK], srckv[:, :, :, 512:SK])
        sync.dma_start(eal_sb[:, 3:8, :, :, :], eald[:, 3:8, :, :, :])
        sync.dma_start(ind4_sb[:, :], ind4[:, :])
        sync.dma_start(sel4_sb[:, :], sel4[:, :])
        sync.dma_start(wop_sb[:, :, :, :], wop[:, :, :, :])
        sync.dma_start(hm_sb[:, :], halom[:, :])
        sync.dma_start(tv_sb[:, :, :], tailv[:, :, :])
        sync.dma_start(tw_sb[:, :, :, :], tailw[:, :, :, :])

        # static ones column of vt (AV denominator row); dummy slots still
        # contribute only exp(-30)~1e-13 to the denominator.
        nc.gpsimd.memset(vt_sb[:, :, :, 16:17], 1.0)

        ones128r = onesr_sb[:, 0:1]
        onesBr = onesr_sb[0:1, :]
        ones128b = onesbf_sb[:, 0:1]
        onesBb = onesbf_sb[0:1, :]

        def tv(pc, i):
            return tv_sb[:, pc, i:i + 1]

        # ================= PHASE 1: prologue + attention =================
        es_pro = ExitStack()
        es_attn = ExitStack()
        pro = es_pro.enter_context(
            tc.tile_pool(name="pro_psum", bufs=2, space="PSUM", side="right"))
        ptmp = es_attn.enter_context(
            tc.tile_pool(name="pro_tmp", bufs=6, side="right"))
        pscore = es_attn.enter_context(
            tc.tile_pool(name="pscore", bufs=2, space="PSUM"))
        ppo = es_attn.enter_context(
            tc.tile_pool(name="ppo", bufs=1, space="PSUM"))
        pexp = es_attn.enter_context(
            tc.tile_pool(name="pexp", bufs=3, side="right"))
        pattn = es_attn.enter_context(
            tc.tile_pool(name="pattn", bufs=2, side="right"))

        po_t = ppo.tile([128, 2, 512], f32, name="po_t", tag="po")
        # zero once: AV writes only 17-row bands; copies read all 128 rows.
        nc.vector.memset(po_t[:, :, :], 0.0)

        # rope broadcast tiles: cs[r, t] = M[., r] rows x cosT/sinT
        def rope_q():
            for v in range(2):
                pbq = pro.tile([128, SQL], f32, name="pbq", tag="pro")
                nc.tensor.matmul(pbq[:, :], rm_sb[:, v, :], rcsq_sb[:, v, :],
                                 start=True, stop=True)
                nc.scalar.activation(csq_sb[:, v, :], pbq[:, :], AF.Copy)

        def rope_k(f0, fw):
            for v in range(2):
                pb = pro.tile([128, 512], f32, name="pb", tag="pro")
                nc.tensor.matmul(pb[:, 0:fw], rm_sb[:, v, :],
                                 rcs_sb[:, v, f0:f0 + fw],
                                 start=True, stop=True)
                nc.scalar.activation(cs_sb[:, v, f0:f0 + fw], pb[:, 0:fw],
                                     AF.Copy)

        def qproj(ch):
            nh = NH[ch]
            te = 96 if ch < 2 else 64   # ts write extent (ch2 keeps 64:70 free)
            for b in range(B):
                pq = pro.tile([96, SQL], f32, name="pq", tag="pro")
                pqs = pro.tile([96, SQL], f32, name="pqs", tag="pro")
                for cc in range(2):
                    nc.tensor.matmul(
                        pq[:, :], wq_sb[:, 0, cc, ch, 0:96],
                        qsrc_sb[:, b, cc, :], start=(cc == 0), stop=(cc == 1))
                    nc.tensor.matmul(
                        pqs[:, :], wq_sb[:, 1, cc, ch, 0:96],
                        qsrc_sb[:, b, cc, :], start=(cc == 0), stop=(cc == 1))
                t1 = ptmp.tile([96, SQL], bf16, name="t1q", tag="ptmp")
                t2 = ptmp.tile([96, SQL], bf16, name="t2q", tag="ptmp")
                nc.vector.scalar_tensor_tensor(
                    t1[:, :], pq[:, :], qkb_sb[0:96, ch, 0:1],
                    csq_sb[0:96, 0, :], op0=OP.add, op1=OP.mult)
                nc.vector.scalar_tensor_tensor(
                    t2[:, :], pqs[:, :], qkb_sb[0:96, ch, 1:2],
                    csq_sb[0:96, 1, :], op0=OP.add, op1=OP.mult)
                nc.gpsimd.tensor_add(qT_sb[0:96, b, ch, :], t1[:, :], t2[:, :])
                for i in range(nh):
                    h = HBASE[ch] + i
                    nc.vector.tensor_scalar(
                        qTv_sb[0:te, h, b, 0:SQL], qT_sb[0:te, b, ch, :],
                        pm_sb[0:te, h:h + 1], None, op0=OP.mult)

        def kproj(ch):
            nh = NH[ch]
            kt = KTT[ch]
            for b in range(B):
                for f0, fw in FCS_CH[ch]:
                    pk = pro.tile([104, 512], f32, name="pk", tag="pro")
                    pks = pro.tile([104, 512], f32, name="pks", tag="pro")
                    for cc in range(2):
                        nc.tensor.matmul(
                            pk[:, 0:fw], wk_sb[:, 0, cc, ch, :],
                            kv_sb[:, b, cc, f0:f0 + fw],
                            start=(cc == 0), stop=(cc == 1))
                        nc.tensor.matmul(
                            pks[:, 0:fw], wk_sb[:, 1, cc, ch, :],
                            kv_sb[:, b, cc, f0:f0 + fw],
                            start=(cc == 0), stop=(cc == 1))
                    t1 = ptmp.tile([104, 512], bf16, name="t1", tag="ptmp")
                    t2 = ptmp.tile([104, 512], bf16, name="t2", tag="ptmp")
                    nc.vector.scalar_tensor_tensor(
                        t1[:, 0:fw], pk[:, 0:fw], qkb_sb[0:104, ch, 2:3],
                        cs_sb[0:104, 0, f0:f0 + fw], op0=OP.add, op1=OP.mult)
                    nc.vector.scalar_tensor_tensor(
                        t2[:, 0:fw], pks[:, 0:fw], qkb_sb[0:104, ch, 3:4],
                        cs_sb[0:104, 1, f0:f0 + fw], op0=OP.add, op1=OP.mult)
                    nc.gpsimd.tensor_add(kt[0:104, b, f0:f0 + fw],
                                         t1[:, 0:fw], t2[:, 0:fw])
                # bias + indicator rows overwrite the zero-padded rows
                kc = KCOLS[ch]
                sync.dma_start(kt[BROW[ch]:BROW[ch] + nh, b, 0:kc],
                               kTbias[b, ch, 0:nh, 0:kc])
                sync.dma_start(kt[BROW[ch] + nh:BROW[ch] + nh + 2, b, 0:kc],
                               kTbias[b, ch, 6:8, 0:kc])

        def vproj(slots):
            for b in range(B):
                for s in slots:
                    hlo = HLO[s]
                    nh_s = 16 - hlo
                    pv = pro.tile([128, 32, 16], f32, name="pv", tag="pro")
                    for cc in range(2):
                        nc.tensor.matmul(pv[:, 0:nh_s, :],
                                         kv_sb[:, b, cc, ts(s, 128)],
                                         wv_sb[:, cc, 16 * hlo:256],
                                         start=(cc == 0), stop=(cc == 1))
                    nc.vector.tensor_copy(
                        vt_sb[:, b, VOFF[s]:VOFF[s] + nh_s, 0:16],
                        pv[:, 0:nh_s, :])

        def attention(pi, b):
            h0 = 2 * pi
            ch = CHUNK_OF_HEAD[h0]
            n = N_P[pi]
            k4 = pi % 4
            kt = KTT[ch]
            K = KCH[ch]
            rhs_t = [None] * n

            def emit_av(si):
                for j in range(2):
                    nc.tensor.matmul(
                        po_t[32 * k4:32 * k4 + 17, j, 0:SQL],
                        vt_sb[:, b, VOFF[si] + (h0 + j - HLO[si]), 0:17],
                        rhs_t[si][:, j, 0:SQL],
                        start=(si == 0), stop=(si == n - 1),
                        skip_group_check=True,
                        tile_position=(0, 32 * k4))

            for si in range(n):
                sc = pscore.tile([128, 2, 512], f32, name="sc", tag="sc")
                for j in range(2):
                    nc.tensor.matmul(
                        sc[:, j, 0:SQL], kt[0:K, b, ts(si, 128)],
                        qTv_sb[0:K, h0 + j, b, 0:SQL],
                        start=True, stop=True)
                ex = pexp.tile([128, 2, SQP], bf16, name="ex", tag="ex")
                nc.scalar.activation(ex[:, :, 0:SQL], sc[:, :, 0:SQL], AF.Exp)
                if KDBG and pi == 2 and b == 1 and si == 1:
                    nc.vector.tensor_copy(dsc_t[:, 0, :, 0:SQL],
                                          sc[:, :, 0:SQL])
                    nc.vector.tensor_copy(dsc_t[:, 1, :, 0:SQL],
                                          ex[:, :, 0:SQL])
                if si in (1, 2):
                    at = pattn.tile([128, 2, SQP], bf16, name="at", tag="at")
                    nc.vector.tensor_mul(at[:, :, 0:SQL], ex[:, :, 0:SQL],
                                         eal_sb[:, pi, si - 1, :, 0:SQL])
                    if KDBG and pi == 2 and b == 1 and si == 1:
                        nc.vector.tensor_copy(dsc_t[:, 2, :, 0:SQL],
                                              at[:, :, 0:SQL])
                    rhs_t[si] = at
                else:
                    rhs_t[si] = ex
                if si >= 1:
                    emit_av(si - 1)
            emit_av(n - 1)

        def close_group(b, g):
            for j in range(2):
                nc.vector.tensor_copy(onum_sb[:, b, g, j, 0:SQL],
                                      po_t[:, j, 0:SQL])

        # ---- emission: prologue chunk-by-chunk, attention b0 then b1 ----
        rope_q()
        rope_k(0, 512)
        qproj(0)
        kproj(0)
        vproj(range(0, 4))
        attention(0, 0)
        attention(1, 0)
        for f0, fw in FCS_CH[2][1:]:
            rope_k(f0, fw)
        qproj(1)
        kproj(1)
        vproj(range(4, 16))
        attention(2, 0)
        attention(3, 0)
        close_group(0, 0)
        qproj(2)
        kproj(2)
        vproj([16])
        # halo mask broadcast [1,SQL] -> [128,SQL]
        pmh = pro.tile([128, SQL], f32, name="pmh", tag="pro")
        nc.tensor.matmul(pmh[:, :], onesBb, hm_sb[:, :], start=True, stop=True)
        nc.scalar.activation(maskB_sb[:, :], pmh[:, :], AF.Copy)
        attention(4, 0)
        attention(5, 0)
        attention(6, 0)
        attention(7, 0)
        close_group(0, 1)
        es_pro.close()
        rm_es.close()

        pdiv = ExitStack()
        pd = pdiv.enter_context(
            tc.tile_pool(name="pdiv", bufs=2, space="PSUM", side="right"))
        pt_es = ExitStack()
        pt = pt_es.enter_context(tc.tile_pool(name="tail_sb", bufs=2))

        def div(b):
            for g in range(2):
                for j in range(2):
                    pden = pd.tile([4, 512], f32, name="pden", tag="a")
                    nc.tensor.matmul(pden[:, 0:SQL], sel4_sb[:, :],
                                     onum_sb[:, b, g, j, 0:SQL],
                                     start=True, stop=True)
                    nc.vector.tensor_copy(denf4[0:4, b, g, j, 0:SQL],
                                          pden[0:4, 0:SQL])
            for g in range(2):
                nc.vector.reciprocal_approx_fast(recf4[0:4, b, g, :, 0:SQL],
                                                 denf4[0:4, b, g, :, 0:SQL])
                nc.vector.tensor_copy(rec4[0:4, b, g, :, 0:SQL],
                                      recf4[0:4, b, g, :, 0:SQL])
            for g in range(2):
                for j in range(2):
                    recB = pd.tile([128, 512], f32, name="recB", tag="a")
                    nc.tensor.matmul(
                        recB[:, 0:SQL], ind4_sb[0:4, :],
                        rec4[0:4, b, g, j, 0:SQL],
                        start=True, stop=True)
                    nc.vector.tensor_mul(oall_sb[:, b, g, j, 0:SQL],
                                         onum_sb[:, b, g, j, 0:SQL],
                                         recB[:, 0:SQL])

        def oproj(b):
            x1p = []
            for pc in range(2):
                px = pd.tile([128, 512], f32, name="px", tag="a")
                for g in range(2):
                    for j in range(2):
                        nc.tensor.matmul(
                            px[:, 0:SQL], wop_sb[:, g, j, ts(pc, 128)],
                            oall_sb[:, b, g, j, 0:SQL],
                            start=(g == 0 and j == 0),
                            stop=(g == 1 and j == 1))
                xt = pt.tile([128, SQL], bf16, name="x1p", tag="x1p")
                nc.vector.scalar_tensor_tensor(
                    xt[:, :], px[:, 0:SQL], tv(pc, 0), qsrc_sb[:, b, pc, :],
                    op0=OP.add, op1=OP.add)
                x1p.append(xt)
            return x1p

        def newton_rsqrt(vee, F):
            """rstd [1,F] f32r from vee [1,F] f32 via bit-trick + 2 Newton."""
            ti = pt.tile([1, SQL], i32, name="nr_ti", tag="nr_i")
            nc.vector.tensor_scalar(ti[:, 0:F], vee.bitcast(i32), 1, None,
                                    op0=OP.logical_shift_right)
            tn = pt.tile([1, SQL], i32, name="nr_tn", tag="nr_f")
            nc.vector.tensor_scalar(tn[:, 0:F], ti[:, 0:F], -1, None,
                                    op0=OP.bitwise_xor)
            y0b = pt.tile([1, SQL], i32, name="nr_y0", tag="nr_i")
            nc.vector.tensor_scalar(y0b[:, 0:F], tn[:, 0:F], 0x5f3759e0, None,
                                    op0=OP.add)
            y = y0b[:, 0:F].bitcast(f32)
            for it in range(2):
                t = pt.tile([1, SQL], f32, name="nr_t", tag="nr_f")
                nc.vector.tensor_mul(t[:, 0:F], y, y)
                h = pt.tile([1, SQL], f32, name="nr_h", tag="nr_f")
                nc.vector.scalar_tensor_tensor(h[:, 0:F], vee, -0.5, t[:, 0:F],
                                               op0=OP.mult, op1=OP.mult)
                yn = pt.tile([1, SQL], f32r if it == 1 else f32,
                             name="nr_y", tag="nr_y")
                nc.vector.scalar_tensor_tensor(yn[:, 0:F], h[:, 0:F], 1.5, y,
                                               op0=OP.add, op1=OP.mult)
                y = yn[:, 0:F]
            return y

        def layernorm(xin, F, gi, bi, pool, odt=bf16, sq_act=False,
                      otag="ln_o", obufs=2):
            """LN over channel dim (256 = partitions across 2 chunks)."""
            sqs = []
            for pc in range(2):
                sq = pt.tile([128, SQL], bf16, name="ln_sq", tag="ln_sq")
                if sq_act:
                    nc.scalar.activation(sq[:, 0:F], xin[pc][:, 0:F], AF.Square)
                else:
                    nc.gpsimd.tensor_mul(sq[:, 0:F], xin[pc][:, 0:F],
                                         xin[pc][:, 0:F])
                sqs.append(sq)
            ps = pool.tile([1, 512], f32, name="ln_ps", tag="a", bufs=2)
            for pc in range(2):
                nc.tensor.matmul(ps[:, 0:F], ones128b, xin[pc][:, 0:F],
                                 start=(pc == 0), stop=(pc == 1))
            ps2 = pool.tile([1, 512], f32, name="ln_ps2", tag="a", bufs=2)
            for pc in range(2):
                nc.tensor.matmul(ps2[:, 0:F], ones128b, sqs[pc][:, 0:F],
                                 start=(pc == 0), stop=(pc == 1))
            mean = pt.tile([1, SQL], f32r, name="ln_mean", tag="ln_mean")
            nc.vector.tensor_scalar(mean[:, 0:F], ps[:, 0:F], 1.0 / D, None,
                                    op0=OP.mult)
            em2 = pt.tile([1, SQL], f32, name="ln_em2", tag="ln_em2")
            nc.vector.scalar_tensor_tensor(em2[:, 0:F], mean[:, 0:F], -1.0,
                                           mean[:, 0:F], op0=OP.mult,
                                           op1=OP.mult)
            var = pt.tile([1, SQL], f32, name="ln_var", tag="ln_var")
            nc.vector.scalar_tensor_tensor(var[:, 0:F], ps2[:, 0:F], 1.0 / D,
                                           em2[:, 0:F], op0=OP.mult, op1=OP.add)
            vee = pt.tile([1, SQL], f32, name="ln_vee", tag="ln_vee")
            nc.vector.tensor_scalar(vee[:, 0:F], var[:, 0:F], 1e-5, None,
                                    op0=OP.add)
            rstd = newton_rsqrt(vee[:, 0:F], F)
            pmb = pool.tile([128, 512], f32, name="ln_pmb", tag="a", bufs=2)
            nc.tensor.matmul(pmb[:, 0:F], onesBr, mean[:, 0:F],
                             start=True, stop=True)
            prb = pool.tile([128, 512], f32, name="ln_prb", tag="a", bufs=2)
            nc.tensor.matmul(prb[:, 0:F], onesBr, rstd, start=True, stop=True)
            outs = []
            for pc in range(2):
                t = pt.tile([128, SQL], bf16, name="ln_t", tag="ln_t")
                nc.vector.tensor_sub(t[:, 0:F], xin[pc][:, 0:F], pmb[:, 0:F])
                t2 = pt.tile([128, SQL], bf16, name="ln_t2", tag="ln_t2")
                nc.vector.tensor_mul(t2[:, 0:F], t[:, 0:F], prb[:, 0:F])
                # (both read PSUM -> must stay on DVE)
                o = pt.tile([128, SQL], odt, name="ln_o", tag=otag,
                            bufs=obufs)
                nc.scalar.activation(o[:, 0:F], t2[:, 0:F], AF.Identity,
                                     bias=tv(pc, bi), scale=tv(pc, gi))
                outs.append(o)
            return outs

        # b0 attention done; div/oproj/LN1/LN2 of b0 overlap b1 attention.
        div(0)
        x1p0 = oproj(0)
        attention(0, 1)
        attention(1, 1)
        x1_0 = layernorm(x1p0, SQL, 1, 2, pd, otag="x1")
        attention(2, 1)
        attention(3, 1)
        close_group(1, 0)
        c0_0 = layernorm(x1_0, SQL, 3, 4, pd, odt=f32r, otag="c0",
                         obufs=4)
        attention(4, 1)
        attention(5, 1)
        attention(6, 1)
        attention(7, 1)
        close_group(1, 1)
        div(1)
        x1p1 = oproj(1)

        # attention fully done: free score/po PSUM, open the wide tail pool
        es_attn.close()
        pt3_es = ExitStack()
        pt3 = pt3_es.enter_context(
            tc.tile_pool(name="tail_psum", bufs=4, space="PSUM"))

        x1_1 = layernorm(x1p1, SQL, 1, 2, pt3, sq_act=True, otag="x1")
        c0_1 = layernorm(x1_1, SQL, 3, 4, pt3, odt=f32r, sq_act=True,
                         otag="c0", obufs=4)
        c0 = {0: c0_0, 1: c0_1}

        def mm4(widx, rhs_tiles, F, pool):
            outs = []
            for pc in range(2):
                p = pool.tile([128, 512], f32, name="mm4", tag="t")
                for cc in range(2):
                    nc.tensor.matmul(p[:, 0:F],
                                     tw_sb[:, widx, cc, ts(pc, 128)],
                                     rhs_tiles[cc][:, 0:F],
                                     start=(cc == 0), stop=(cc == 1))
                outs.append(p)
            return outs

        # pw1 + gelu (first gelu-set op; single table switch lives here)
        cm = {}
        for b in range(B):
            cp = mm4(1, c0[b], SQL, pt3)
            cmb = []
            for pc in range(2):
                cg = pt.tile([128, SQL], bf16, name="cg", tag="cg")
                nc.scalar.activation(cg[:, :], cp[pc][:, 0:SQL], AF.Gelu,
                                     bias=tv(pc, 5))
                cmt = pt.tile([128, SQL], bf16, name="cmt", tag="cmt", bufs=4)
                nc.gpsimd.tensor_mul(cmt[:, :], cg[:, :], maskB_sb[:, :])
                cmb.append(cmt)
            cm[b] = cmb
        # depthwise conv (3 taps) + BN + hardswish
        hsw = {}
        for b in range(B):
            hswb = []
            for pc in range(2):
                cmp_ = cm[b][pc]
                a1 = pt.tile([128, WIN], bf16, name="a1", tag="a1")
                nc.vector.tensor_scalar(a1[:, :], cmp_[:, 1:WIN + 1],
                                        tv(pc, 7), None, op0=OP.mult)
                a2 = pt.tile([128, WIN], bf16, name="a2", tag="a2")
                nc.vector.scalar_tensor_tensor(
                    a2[:, :], cmp_[:, 0:WIN], tv(pc, 6), a1[:, :],
                    op0=OP.mult, op1=OP.add)
                a3 = pt.tile([128, WIN], bf16, name="a3", tag="a3")
                nc.vector.scalar_tensor_tensor(
                    a3[:, :], cmp_[:, 2:WIN + 2], tv(pc, 8), a2[:, :],
                    op0=OP.mult, op1=OP.add)
                bn = pt.tile([128, WIN], bf16, name="bn", tag="bn")
                nc.scalar.activation(bn[:, :], a3[:, :], AF.Identity,
                                     bias=tv(pc, 10), scale=tv(pc, 9))
                h1 = pt.tile([128, WIN], bf16, name="h1", tag="h1")
                nc.vector.tensor_scalar(h1[:, :], bn[:, :], 3.0, 6.0,
                                        op0=OP.add, op1=OP.min)
                h2 = pt.tile([128, WIN], bf16, name="h2", tag="h2")
                nc.scalar.activation(h2[:, :], h1[:, :], AF.Relu,
                                     scale=1.0 / 6.0)
                hst = pt.tile([128, WIN], f32r, name="hst", tag="hst", bufs=4)
                nc.gpsimd.tensor_mul(hst[:, :], bn[:, :], h2[:, :])
                hswb.append(hst)
            hsw[b] = hswb
        # pw2 (+bias) then FFN
        x2 = {}
        for b in range(B):
            p2 = mm4(2, hsw[b], WIN, pt3)
            x2b = []
            for pc in range(2):
                x2t = pt.tile([128, WIN], f32r, name="x2t", tag="x2t", bufs=4)
                nc.scalar.activation(x2t[:, :], p2[pc][:, 0:WIN], AF.Identity,
                                     bias=tv(pc, 11))
                x2b.append(x2t)
            x2[b] = x2b
        gg = {}
        for b in range(B):
            p3 = mm4(3, x2[b], WIN, pt3)
            ggb = []
            for pc in range(2):
                g1 = pt.tile([128, WIN], f32r, name="g1", tag="g1", bufs=4)
                nc.scalar.activation(g1[:, :], p3[pc][:, 0:WIN], AF.Gelu,
                                     bias=tv(pc, 12))
                ggb.append(g1)
            gg[b] = ggb
        for b in range(B):
            p4 = mm4(4, gg[b], WIN, pt3)
            x3 = []
            for pc in range(2):
                x3t = pt.tile([128, WIN], bf16, name="x3t", tag="x3t")
                nc.vector.scalar_tensor_tensor(
                    x3t[:, :], p4[pc][:, 0:WIN], tv(pc, 13), x2[b][pc][:, :],
                    op0=OP.add, op1=OP.add)
                x3.append(x3t)
            xo = layernorm(x3, WIN, 14, 15, pt3, odt=f32, sq_act=True,
                           otag="xo")
            for pc in range(2):
                sync.dma_start(out[b, ts(pc, 128), :], xo[pc][:, 0:WIN])

        if KDBG:
            sync.dma_start(d_onum[:, :, :, :, :], onum_sb[:, :, :, :, :])
            sync.dma_start(d_oall[:, :, :, :, :], oall_sb[:, :, :, :, :])
            sync.dma_start(d_den[:, :, :, :], den36[:, :, :, :])
            sync.dma_start(d_rec[:, :, :, :], recf36[:, :, :, :])
            sync.dma_start(d_qtv[:, :, :, :], qTv_sb[:, :, :, :])
            sync.dma_start(d_kt0[:, :, :], kT0[:, :, :])
            sync.dma_start(d_vt[:, :, :, :], vt_sb[:, :, :, :])
            sync.dma_start(d_eal[:, :, :, :, :], eal_sb[:, :, :, :, :])
            sync.dma_start(d_sc[:, :, :, :], dsc_t[:, :, :, :])
        pdiv.close()
        pt3_es.close()
        pt_es.close()
        for _t, _free in reversed(_keep):
            _free()

    nc.compile()
    return nc


# ---------------------------------------------------------------------------
# Host-side input prep (sharding)
# ---------------------------------------------------------------------------

def host_prep(inputs):
    f32 = np.float32

    src = np.asarray(inputs["src"], f32)
    alibi = np.asarray(inputs["alibi_bias"], f32)
    pos_emb = np.asarray(inputs["pos_emb"], f32)
    mask = np.asarray(inputs["mask"])

    slopes = -alibi[:, 0, 1].astype(np.float64)  # alibi[h,0,1] = -slope_h
    exp_slopes = 2.0 ** (-8.0 * (np.arange(H) + 1) / H)
    assert np.allclose(slopes, exp_slopes, rtol=1e-3), "unexpected alibi slopes"

    cos = np.cos(pos_emb).astype(f32)  # [S, 8]
    sin = np.sin(pos_emb).astype(f32)

    # swap perm within each head: j -> (j+8)%16
    jj = np.arange(D)
    swap = (jj // HD) * HD + (jj % HD + HD // 2) % HD

    wq, wk, wvm = [np.asarray(inputs[k], f32) for k in ("wq", "wk", "wv")]
    bq, bk, bv = [np.asarray(inputs[k], f32) for k in ("bq", "bk", "bv")]

    # chunked weight cols, zero-padded to 104 (defines kT/qT rows 96:104)
    def chunk_cols(m):  # m [D, D] -> [D, 3, 104]
        outm = np.zeros((D, 3, 104), f32)
        for ch in range(3):
            w = 16 * NH[ch]
            outm[:, ch, 0:w] = m[:, 96 * ch:96 * ch + w]
        return outm

    wq2 = np.stack([chunk_cols(wq.T), chunk_cols(wq.T[:, swap])]).astype(BF16)
    wk2 = np.stack([chunk_cols(wk.T), chunk_cols(wk.T[:, swap])]).astype(BF16)

    # qkb [128, 3, 4]: chunk-local rows
    qkb = np.zeros((128, 3, 4), f32)
    for ch in range(3):
        w = 16 * NH[ch]
        sl = slice(96 * ch, 96 * ch + w)
        qkb[0:w, ch, 0] = bq[sl]
        qkb[0:w, ch, 1] = bq[swap][sl]
        qkb[0:w, ch, 2] = bk[sl]
        qkb[0:w, ch, 3] = bk[swap][sl]

    # Mcos[i, v, r]: cos-select (r%8==i); Msin adds sign by half
    r = np.arange(128)
    mc = (r[None, :] % 8 == np.arange(8)[:, None]).astype(f32)
    sgn_r = np.where((r % HD) < HD // 2, -1.0, 1.0).astype(f32)
    ropem = np.ascontiguousarray(
        np.stack([mc, mc * sgn_r[None, :]], 1)).astype(BF16)

    # head-slot row masks [128, 16]
    pm = np.zeros((128, 16), f32)
    for h in range(16):
        ch = CHUNK_OF_HEAD[h]
        i = h - HBASE[ch]
        pm[16 * i:16 * i + 16, h] = 1.0

    # qTv bias rows [8, 16, 2, SQL] (b-duplicated for one-DMA load)
    fidx = np.arange(SQL, dtype=f32)
    qvr = np.zeros((8, 16, SQL), f32)
    for h in range(16):
        ch = CHUNK_OF_HEAD[h]
        nh = NH[ch]
        i = h - HBASE[ch]
        sl_f = np.float32(slopes[h])
        qvr[i, h, :] = 1.0
        qvr[nh, h, :] = (-sl_f * fidx).astype(BF16).astype(f32)
        qvr[nh + 1, h, :] = (-sl_f * (257.0 - fidx)).astype(BF16).astype(f32)
    qvr = np.repeat(qvr[:, :, None, :], 2, axis=2).astype(BF16)

    # tail weights / vectors
    wo, pw1, pw2, w1m, w2m = [np.asarray(inputs[k], f32)
                              for k in ("wo", "pw1_w", "pw2_w", "w1", "w2")]
    tailw = np.stack([wo.T, pw1.T, pw2.T, w1m.T, w2m.T])  # [5, D, D]
    tailw = np.ascontiguousarray(
        tailw.reshape(5, 2, 128, D).transpose(2, 0, 1, 3)).astype(f32)
    dww = np.asarray(inputs["dw_w"], f32)  # [D, 1, 3]
    sbn = (np.asarray(inputs["bn_g"], f32) /
           np.sqrt(np.asarray(inputs["bn_var"], f32) + 1e-5))
    tbn = ((np.asarray(inputs["dw_b"], f32) -
            np.asarray(inputs["bn_mean"], f32)) * sbn +
           np.asarray(inputs["bn_b"], f32))
    bo2 = np.asarray(inputs["bo"], f32) + wo @ bv   # vproj bias fold
    vecs = [bo2, inputs["n1_g"], inputs["n1_b"], inputs["ln_g"],
            inputs["ln_b"], inputs["pw1_b"], dww[:, 0, 0], dww[:, 0, 1],
            dww[:, 0, 2], sbn, tbn, inputs["pw2_b"], inputs["b1"],
            inputs["b2"], inputs["n2_g"], inputs["n2_b"]]
    tailv = np.stack([np.asarray(v, f32) for v in vecs], -1)  # [D, 16]
    tailv = np.ascontiguousarray(
        tailv.reshape(2, 128, 16).transpose(1, 0, 2)).astype(f32)

    # wop [2, 2, 128, D]: oall row p of (g, j) -> head 2*(4g+p//32)+j, hd p%32
    wopt = np.zeros((2, 2, 128, D), f32)
    p128 = np.arange(128)
    for g in range(2):
        for j in range(2):
            h_of = 2 * (4 * g + p128 // 32) + j
            hd = p128 % 32
            real = hd < 16
            wopt[g, j, real, :] = wo.T[(h_of * 16 + hd)[real], :]
    wopt = np.ascontiguousarray(wopt.transpose(2, 0, 1, 3))  # [128, 2, 2, D]

    # ind4 [4, 128]: recB[p] = rec[p//32];  sel4 [128, 4]: den row selector
    ind4 = np.zeros((4, 128), f32)
    for rr in range(4):
        ind4[rr, p128 // 32 == rr] = 1.0
    sel4 = np.zeros((128, 4), f32)
    for k in range(4):
        sel4[32 * k + 16, k] = 1.0

    maskvec = mask.astype(bool)  # [B, S]

    in_maps = []
    for c in range(NCORES):
        q0 = c * WIN - 1
        wb = 2 * c - 1
        diag = [wb, wb + 1, wb + 2, wb + 3]

        def mindist(g):
            return max(q0 - (128 * g + 127), 128 * g - (q0 + 257), 0)

        rest = sorted((g for g in range(NB) if g not in diag), key=mindist)
        perm = [(g if 0 <= g < NB else -1) for g in diag] + rest
        perm = perm + [-1] * (NSLOT - len(perm))

        # permuted / padded per-core tensors
        srckv_c = np.zeros((B, D, SK), f32)
        ropecs_c = np.zeros((8, 2, SK), f32)
        for s, g in enumerate(perm):
            if g < 0:
                continue
            sl = slice(128 * s, 128 * s + 128)
            gsl = slice(128 * g, 128 * g + 128)
            srckv_c[:, :, sl] = src.transpose(0, 2, 1)[:, :, gsl]
            ropecs_c[:, 0, sl] = cos[gsl, :].T
            ropecs_c[:, 1, sl] = sin[gsl, :].T

        # kT bias rows [B, 3, 8, SK]
        kTb = np.zeros((B, 3, 8, SK), f32)
        skpos = np.zeros(SK, np.int64)
        isdum = np.zeros(SK, bool)
        for s, g in enumerate(perm):
            sl = slice(128 * s, 128 * s + 128)
            if g < 0:
                isdum[sl] = True
            else:
                skpos[sl] = 128 * g + p128
        edged = np.maximum(np.maximum(q0 - skpos, skpos - (q0 + 257)), 0
                           ).astype(f32)
        indL = ((skpos <= q0) & ~isdum).astype(f32)
        indR = ((skpos >= q0 + 257) & ~isdum).astype(f32)
        for b in range(B):
            mrow = np.where(maskvec[b][np.clip(skpos, 0, S - 1)], -30.0, 0.0
                            ).astype(f32)
            for ch in range(3):
                for i in range(NH[ch]):
                    h = HBASE[ch] + i
                    bias_row = (-np.float32(slopes[h]) * edged) + mrow
                    bias_row[isdum] = -30.0
                    kTb[b, ch, i, :] = bias_row
                kTb[b, ch, 6, :] = indL
                kTb[b, ch, 7, :] = indR
        kTb_bf = kTb.astype(BF16)

        # diag exp(alibi) for the two fully-in-window slots (perm[1], perm[2])
        eald_c = np.ones((8, 128, 2, 2, SQP), f32)
        qpos = np.clip(q0 + np.arange(SQL), 0, S - 1)
        for pi in range(8):
            for si2 in range(2):
                g = perm[1 + si2]
                sk = 128 * g + p128
                for j in range(2):
                    h = 2 * pi + j
                    eald_c[pi, :, si2, j, 0:SQL] = np.exp(
                        np.minimum(alibi[h][np.ix_(sk, qpos)], 0.0))

        # q-window slice with zero padding
        sq = np.zeros((B, SQL, D), f32)
        lo, hi = max(q0, 0), min(q0 + SQL, S)
        sq[:, lo - q0:hi - q0, :] = src[:, lo:hi, :]
        srcqT = sq.transpose(0, 2, 1)  # [B, D, SQL]

        csq = np.zeros((8, 2, SQL), f32)
        csq[:, 0, lo - q0:hi - q0] = 0.25 * cos[lo:hi, :].T
        csq[:, 1, lo - q0:hi - q0] = 0.25 * sin[lo:hi, :].T

        halo = np.ones((1, SQL), f32)
        if q0 < 0:
            halo[0, 0] = 0.0
        if q0 + SQL > S:
            halo[0, SQL - 1] = 0.0

        wqk_h = np.ascontiguousarray(
            np.stack([wq2, wk2], 0).reshape(2, 2, 2, 128, 3, 104)
            .transpose(3, 0, 1, 2, 4, 5))  # [128, qk, v, cc, ch, 104]
        in_maps.append({
            "srckv": np.ascontiguousarray(
                srckv_c.reshape(B, 2, 128, SK).transpose(2, 0, 1, 3)
                ).astype(BF16),
            "srcq": np.ascontiguousarray(
                srcqT.reshape(B, 2, 128, SQL).transpose(2, 0, 1, 3)
                ).astype(BF16),
            "wqk": wqk_h,
            "wv": np.ascontiguousarray(
                wvm.T.reshape(2, 128, D).transpose(1, 0, 2)).astype(BF16),
            "ropecs": ropecs_c.astype(BF16),
            "ropecsq": csq.astype(BF16),
            "ropem": ropem,
            "qkb": qkb,
            "kTbias": kTb_bf,
            "qvr": qvr,
            "eald": np.ascontiguousarray(
                eald_c.transpose(1, 0, 2, 3, 4)).astype(BF16),
            "tailw": tailw,
            "tailv": tailv,
            "halom": halo.astype(BF16),
            "pmask": pm,
            "onesbf": np.ones((128, 128), BF16),
            "onesr": np.ones((128, 128), f32),
            "wop": wopt.astype(BF16),
            "ind4": ind4,
            "sel4": sel4.astype(BF16),
        })
    return in_maps


def kernel(**inputs) -> np.ndarray:
    _ensure_ntff_hook()
    from concourse.bass_utils import run_bass_kernel_spmd

    if "nc" not in _COMPILED:
        _COMPILED["nc"] = build_nc()
    nc = _COMPILED["nc"]

    in_maps = host_prep(inputs)
    trace = os.environ.get("KERNEL_TRACE", "0") == "1"
    res = run_bass_kernel_spmd(nc, in_maps, core_ids=list(range(NCORES)),
                               trace=trace)
    kernel.last_result = res

    b_, s_, d_ = inputs["src"].shape
    full = np.empty((b_, s_, d_), np.float32)
    for c in range(NCORES):
        o = res.results[c]["out"]  # [B, D, win]
        full[:, c * WIN:(c + 1) * WIN, :] = o.transpose(0, 2, 1)
    return full
